# revision 15
# baseline (speedup 1.0000x reference)
"""Viterbi decode (CRF layer) on Trainium2 — Bass kernel.

Problem: feats [1024, 512, 128] f32, transitions [128, 128],
start/stop_transitions [128] -> best tag sequence [1024, 512] int32.

Strategy: pure batch data-parallelism across 8 NeuronCores. Each core takes
128 batch rows (= 128 SBUF partitions) and runs the sequential max-plus
forward scan on-chip:

    sc[b, i, j] = v[b, i] + trans[i, j]          (fp32, one rounding)
    mx[b, j]    = max_i sc[b, i, j]
    v'[b, j]    = mx[b, j] + feats[b, t, j]      (fp32, one rounding)

The per-step state vectors v stream to DRAM; the backtrace recomputes the
argmax only along the traced path (B*S tiny argmaxes) on host during the
unshard step, with identical fp32 arithmetic and first-index tie-breaking,
so the final int32 tags match the reference bit-exactly.

variant="v0" keeps the full device-side backpointer computation (slower,
fully self-contained backpointers) as a fallback.
"""

import numpy as np

B, S, T = 1024, 512, 128
NCORES = 8
BL = B // NCORES  # 128 batch rows per core == SBUF partition count


def build_viterbi_nc(trans_np, S_=S, T_=T, BL_=BL, variant="v1"):
    """Build the per-core Bass program (same NEFF for all cores).

    NOTE: start_transitions must already be folded into feats[:, 0, :] by the
    caller (bit-exact: same single fp32 add the reference performs).

    walrus/core_v3 allows only ONE attached sync-wait per compute
    instruction; the initial state goes through a DVE tensor_copy so every
    instruction waits on at most one foreign semaphore.
    """
    import concourse.bacc as bacc
    import concourse.mybir as mybir
    import concourse.tile as tile

    f32 = mybir.dt.float32
    add = mybir.AluOpType.add
    mx_op = mybir.AluOpType.max
    eq_op = mybir.AluOpType.is_equal
    mul_op = mybir.AluOpType.mult
    X = mybir.AxisListType.X

    nc = bacc.Bacc("TRN2", target_bir_lowering=False, debug=False)
    feats = nc.declare_dram_parameter("feats", [BL_, S_, T_], f32, isOutput=False)
    if variant == "v0":
        bp = nc.declare_dram_parameter("bp", [S_ - 1, BL_, T_], f32, isOutput=True)
    else:
        vs_out = nc.declare_dram_parameter("vs", [S_ - 1, BL_, T_], f32, isOutput=True)
    v_final = nc.declare_dram_parameter("v_final", [BL_, T_], f32, isOutput=True)

    if variant == "v2":
        # table stored [j, i] (transposed) so the score buffer is written and
        # reduced fully contiguously in [b, j, i] order
        tbl = np.ascontiguousarray(trans_np.T.reshape(1, T_ * T_), dtype=np.float32)
    else:
        tbl = np.ascontiguousarray(trans_np.reshape(1, T_ * T_), dtype=np.float32)
    tbc_d = nc.inline_tensor(tbl, "tbc")
    iota_d = nc.inline_tensor(
        np.arange(T_ - 1, -1, -1, dtype=np.float32).reshape(1, T_), "iotad"
    )

    with tile.TileContext(nc) as tc:
        with (
            tc.tile_pool(name="const", bufs=1) as cpool,
            tc.tile_pool(name="feat", bufs=8) as fpool,
            tc.tile_pool(name="vst", bufs=4) as vpool,
            tc.tile_pool(name="sc", bufs=1 if variant == "v0" else 2) as scpool,
            tc.tile_pool(name="mx", bufs=2) as mxpool,
            tc.tile_pool(name="bpp", bufs=4) as bppool,
        ):
            tbc = cpool.tile([BL_, T_ * T_], f32, tag="tbc")
            nc.gpsimd.dma_start(tbc[:, :], tbc_d[:, :].partition_broadcast(BL_))
            iotab = cpool.tile([BL_, T_], f32, tag="iotab")
            nc.gpsimd.dma_start(iotab[:, :], iota_d[:, :].partition_broadcast(BL_))

            f0 = fpool.tile([BL_, T_], f32, tag="feat")
            nc.gpsimd.dma_start(f0[:, :], feats[:, 0, :])
            v = vpool.tile([BL_, T_], f32, tag="v")
            nc.vector.tensor_copy(v[:, :], f0[:, :])

            tb3 = tbc[:, :].rearrange("p (i j) -> p i j", i=T_)
            io3 = iotab[:, :].unsqueeze(-1).broadcast_to([BL_, T_, T_])
            # v2: table is [j, i]-major; split the add by j between DVE and
            # Pool (Pool ~2x slower -> give it the smaller range)
            import os as _os
            JSPLIT = int(_os.environ.get("VT_JSPLIT", T_))
            DSPLIT = int(_os.environ.get("VT_DSPLIT", T_ // 2))

            for t in range(1, S_):
                ft = fpool.tile([BL_, T_], f32, tag="feat")
                nc.gpsimd.dma_start(ft[:, :], feats[:, t, :])

                sc = scpool.tile([BL_, T_ * T_], f32, tag="sc")
                sc3 = sc[:, :].rearrange("p (i j) -> p i j", i=T_)
                scT = sc[:, :].rearrange("p (i j) -> p j i", i=T_)
                mxt = mxpool.tile([BL_, T_], f32, tag="mx")

                if variant == "v2":
                    # sc[b, j, i] = v[b, i] + tT[j, i]; contiguous writes
                    scJ = sc[:, :].rearrange("p (j i) -> p j i", j=T_)
                    tbJ = tbc[:, :].rearrange("p (j i) -> p j i", j=T_)
                    nA = JSPLIT * T_
                    v3a = v[:, :].unsqueeze(1).broadcast_to([BL_, JSPLIT, T_])
                    scA = sc[:, 0:nA].rearrange("p (j i) -> p j i", j=JSPLIT)
                    tbA = tbc[:, 0:nA].rearrange("p (j i) -> p j i", j=JSPLIT)
                    nc.vector.tensor_tensor(scA, v3a, tbA, add)
                    if JSPLIT < T_:
                        v3b = v[:, :].unsqueeze(1).broadcast_to(
                            [BL_, T_ - JSPLIT, T_]
                        )
                        scB = sc[:, nA : T_ * T_].rearrange(
                            "p (j i) -> p j i", j=T_ - JSPLIT
                        )
                        tbB = tbc[:, nA : T_ * T_].rearrange(
                            "p (j i) -> p j i", j=T_ - JSPLIT
                        )
                        nc.gpsimd.tensor_tensor(scB, v3b, tbB, add)
                    nc.vector.tensor_reduce(mxt[:, :], scJ, axis=X, op=mx_op)
                elif variant == "v3":
                    # sc[b,i,j] = t[i,j] + v[b,i]: DVE does rows [0, DSPLIT)
                    # in one tensor_tensor; ACT does rows [DSPLIT, T) as
                    # per-row activation-adds (bias = per-partition scalar)
                    nD = DSPLIT * T_
                    v3a = v[:, 0:DSPLIT].unsqueeze(-1).broadcast_to(
                        [BL_, DSPLIT, T_]
                    )
                    scA = sc[:, 0:nD].rearrange("p (i j) -> p i j", i=DSPLIT)
                    tbA = tbc[:, 0:nD].rearrange("p (i j) -> p i j", i=DSPLIT)
                    nc.vector.tensor_tensor(scA, v3a, tbA, add)
                    for i in range(DSPLIT, T_):
                        nc.scalar.add(
                            sc[:, i * T_ : (i + 1) * T_],
                            tbc[:, i * T_ : (i + 1) * T_],
                            v[:, i : i + 1],
                        )
                    nc.vector.tensor_reduce(mxt[:, :], scT, axis=X, op=mx_op)
                else:
                    v3 = v[:, :].unsqueeze(-1).broadcast_to([BL_, T_, T_])
                    nc.vector.tensor_tensor(sc3, v3, tb3, add)
                    nc.vector.tensor_reduce(mxt[:, :], scT, axis=X, op=mx_op)

                vn = vpool.tile([BL_, T_], f32, tag="v")
                nc.vector.tensor_tensor(vn[:, :], mxt[:, :], ft[:, :], add)

                if variant == "v0":
                    # backpointers on device: sc <- (sc==mx)*(T-1-i); bp=max_i
                    mx3 = mxt[:, :].unsqueeze(1).broadcast_to([BL_, T_, T_])
                    nc.vector.tensor_tensor(sc3, sc3, mx3, eq_op)
                    nc.vector.tensor_tensor(sc3, sc3, io3, mul_op)
                    bpt = bppool.tile([BL_, T_], f32, tag="bp")
                    nc.vector.tensor_reduce(bpt[:, :], scT, axis=X, op=mx_op)
                    nc.gpsimd.dma_start(bp[t - 1, :, :], bpt[:, :])
                else:
                    nc.gpsimd.dma_start(vs_out[t - 1, :, :], vn[:, :])

                v = vn

            nc.gpsimd.dma_start(v_final[:, :], v[:, :])
    nc.finalize()
    return nc


def build_viterbi_f16_nc(trans_np, S_=S, T_=T, BL_=BL, kblk=8):
    """fp16 forward-scan kernel: per step, the [T,T] score add and the
    max-tree run in fp16 on DVE (4x perf mode); the state update
    vn = max + feat stays fp32, with per-step recentring (subtract the
    per-row max) so fp16 magnitudes stay ~|8|. The recentred fp32 state
    trajectory streams to DRAM b-major; the host backtraces from it in fp32.

    Numerics validated against reference in numpy sim: ~1e-4 tag mismatch
    rate (rel err ~7e-3, gate is 2e-2).
    """
    import concourse.bacc as bacc
    import concourse.mybir as mybir
    import concourse.tile as tile

    f32 = mybir.dt.float32
    f16 = mybir.dt.float16
    add = mybir.AluOpType.add
    mx_op = mybir.AluOpType.max
    mul_op = mybir.AluOpType.mult
    sub_op = mybir.AluOpType.subtract
    X = mybir.AxisListType.X

    nc = bacc.Bacc("TRN2", target_bir_lowering=False, debug=False)
    feats = nc.declare_dram_parameter("feats", [BL_, S_, T_], f32, isOutput=False)
    vs_out = nc.declare_dram_parameter("vs", [BL_, S_ - 1, T_], f32, isOutput=True)

    # table stored [j, i] (transposed) so score writes and the i-tree are
    # contiguous per j
    tbl16 = np.ascontiguousarray(trans_np.T.reshape(1, T_ * T_)).astype(np.float16)
    tbc_d = nc.inline_tensor(tbl16, "tbc16")

    nblk = (S_ + kblk - 1) // kblk  # feat blocks cover s in [0, S)

    with tile.TileContext(nc) as tc:
        with (
            tc.tile_pool(name="const", bufs=1) as cpool,
            tc.tile_pool(name="feat", bufs=2) as fpool,
            tc.tile_pool(name="vsb", bufs=2) as vspool,
            tc.tile_pool(name="sc", bufs=1) as scpool,
            tc.tile_pool(name="small", bufs=2) as smpool,
        ):
            tbc = cpool.tile([BL_, T_ * T_], f16, tag="tbc")
            nc.gpsimd.dma_start(tbc[:, :], tbc_d[:, :].partition_broadcast(BL_))
            t3 = tbc[:, :].rearrange("p (j i) -> p j i", j=T_)

            s16 = scpool.tile([BL_, T_ * T_], f16, tag="s16")
            s3 = s16[:, :].rearrange("p (j i) -> p j i", j=T_)

            # feat block 0 (s = 0..kblk-1)
            fb = fpool.tile([BL_, kblk * T_], f32, tag="fb")
            nc.gpsimd.dma_start(
                fb[:, :].rearrange("p (k t) -> p k t", k=kblk), feats[:, 0:kblk, :]
            )

            # initial state from f0 (host already folded start_transitions)
            f0 = fb[:, 0:T_]
            shift = smpool.tile([BL_, 1], f32, tag="shift")
            nc.vector.tensor_reduce(shift[:, :], f0, axis=X, op=mx_op)
            v16 = smpool.tile([BL_, T_], f16, tag="v16")
            nc.vector.tensor_scalar(v16[:, :], f0, shift[:, :], None, sub_op)

            vsb = vspool.tile([BL_, kblk * T_], f32, tag="vsb")

            for t in range(1, S_):
                kf, rf = divmod(t, kblk)
                if rf == 0:  # need next feat block (covers s = t..t+kblk-1)
                    fb = fpool.tile([BL_, kblk * T_], f32, tag="fb")
                    hi = min(kblk, S_ - kf * kblk)
                    nc.gpsimd.dma_start(
                        fb[:, 0 : hi * T_].rearrange("p (k t) -> p k t", k=hi),
                        feats[:, kf * kblk : kf * kblk + hi, :],
                    )
                ft = fb[:, rf * T_ : (rf + 1) * T_]

                # s16[b,j,i] = fp16(v16[b,i] + t16[j,i])   (4x DVE mode)
                v3 = v16[:, :].unsqueeze(1).broadcast_to([BL_, T_, T_])
                nc.vector.scalar_tensor_tensor(s3, v3, 1.0, t3, mul_op, add)

                # in-place max tree over i: 128 -> 1
                w = T_ // 2
                while w >= 1:
                    a = s3[:, :, 0:w]
                    b = s3[:, :, w : 2 * w]
                    if w > 1:
                        nc.vector.scalar_tensor_tensor(a, a, 1.0, b, mul_op, mx_op)
                    else:
                        mx16 = smpool.tile([BL_, T_], f16, tag="mx16")
                        m3 = mx16[:, :].rearrange("p (j i) -> p j i", j=T_, i=1)
                        nc.vector.scalar_tensor_tensor(m3, a, 1.0, b, mul_op, mx_op)
                    w //= 2

                # vn32 = fp32(mx16) + ft   -> written into the vs block slot
                r = (t - 1) % kblk
                if r == 0:
                    vsb = vspool.tile([BL_, kblk * T_], f32, tag="vsb")
                vslot = vsb[:, r * T_ : (r + 1) * T_]
                nc.vector.scalar_tensor_tensor(vslot, mx16[:, :], 1.0, ft, mul_op, add)

                # recentre: shift = max_j vn; v16 = fp16(vn - shift)
                shift = smpool.tile([BL_, 1], f32, tag="shift")
                nc.vector.tensor_reduce(shift[:, :], vslot, axis=X, op=mx_op)
                v16 = smpool.tile([BL_, T_], f16, tag="v16")
                nc.vector.tensor_scalar(v16[:, :], vslot, shift[:, :], None, sub_op)

                if r == kblk - 1 or t == S_ - 1:  # flush vs block
                    lo = (t - 1) - r  # first vs row in this block
                    n = r + 1
                    nc.gpsimd.dma_start(
                        vs_out[:, lo : lo + n, :],
                        vsb[:, 0 : n * T_].rearrange("p (k t) -> p k t", k=n),
                    )
    nc.finalize()
    return nc


def build_viterbi_v4_nc(trans_np, S_=S, T_=T, BL_=BL, kblk=8, a_rows=58,
                        apad=64):
    """3-engine fp32 kernel. Exact arithmetic (same single-rounding adds as
    the reference), so tags match bit-exactly.

    Score add s[b,i,j] = t[i,j] + v[b,i] split by i-rows:
      - Act: rows [0, a)   -> s_A buffer, [i, j] layout (row-contig), one
        activation-add per row (bias = v[:, i]). Rows [a, apad) are -1e38
        pad written once so DVE can run a fixed power-of-2 max tree.
      - Pool: rows [a, T)  -> s_P buffer in compact TRANSPOSED [j, k] layout
        (one tensor_tensor add; Pool has no max op, DVE reduces contiguous).
    DVE: in-place contiguous max tree over s_A rows (apad -> 1), one
    contiguous tensor_reduce over s_P, combine, + feat -> vs block slot.
    State trajectory streams b-major; host does the exact fp32 backtrace.
    """
    import concourse.bacc as bacc
    import concourse.mybir as mybir
    import concourse.tile as tile

    f32 = mybir.dt.float32
    add = mybir.AluOpType.add
    mx_op = mybir.AluOpType.max
    mul_op = mybir.AluOpType.mult
    X = mybir.AxisListType.X

    p_rows = T_ - a_rows
    assert a_rows <= apad and (apad & (apad - 1)) == 0

    nc = bacc.Bacc("TRN2", target_bir_lowering=False, debug=False)
    feats = nc.declare_dram_parameter("feats", [BL_, S_, T_], f32, isOutput=False)
    vs_out = nc.declare_dram_parameter("vs", [BL_, S_ - 1, T_], f32, isOutput=True)

    # Act table: rows [0, a) of trans, [i, j] layout
    tblA = np.ascontiguousarray(trans_np[0:a_rows, :].reshape(1, a_rows * T_),
                                dtype=np.float32)
    tblA_d = nc.inline_tensor(tblA, "tblA")
    # Pool table: rows [a, T) transposed-compact: tP[j, k] = trans[a+k, j]
    tblP = np.ascontiguousarray(trans_np[a_rows:, :].T.reshape(1, T_ * p_rows),
                                dtype=np.float32)
    tblP_d = nc.inline_tensor(tblP, "tblP")

    with tile.TileContext(nc) as tc:
        with (
            tc.tile_pool(name="const", bufs=1) as cpool,
            tc.tile_pool(name="feat", bufs=2) as fpool,
            tc.tile_pool(name="vsb", bufs=2) as vspool,
            tc.tile_pool(name="sc", bufs=1) as scpool,
            tc.tile_pool(name="small", bufs=2) as smpool,
        ):
            tbA = cpool.tile([BL_, a_rows * T_], f32, tag="tbA")
            nc.gpsimd.dma_start(tbA[:, :], tblA_d[:, :].partition_broadcast(BL_))
            tbP = cpool.tile([BL_, T_ * p_rows], f32, tag="tbP")
            nc.gpsimd.dma_start(tbP[:, :], tblP_d[:, :].partition_broadcast(BL_))
            tbP3 = tbP[:, :].rearrange("p (j k) -> p j k", j=T_)

            sA = scpool.tile([BL_, apad * T_], f32, tag="sA")
            sA3 = sA[:, :].rearrange("p (i j) -> p i j", i=apad)
            if a_rows < apad:  # one-time -inf pad rows for the fixed tree
                nc.vector.memset(sA[:, a_rows * T_ :], -1.0e38)
            sP = scpool.tile([BL_, T_ * p_rows], f32, tag="sP")
            sP3 = sP[:, :].rearrange("p (j k) -> p j k", j=T_)

            fb = fpool.tile([BL_, kblk * T_], f32, tag="fb")
            nc.gpsimd.dma_start(
                fb[:, :].rearrange("p (k t) -> p k t", k=kblk), feats[:, 0:kblk, :]
            )
            v = fb[:, 0:T_]  # v_0 = feats[:,0] (start folded by host)

            vsb = vspool.tile([BL_, kblk * T_], f32, tag="vsb")

            for t in range(1, S_):
                kf, rf = divmod(t, kblk)
                if rf == 0:
                    fb = fpool.tile([BL_, kblk * T_], f32, tag="fb")
                    hi = min(kblk, S_ - kf * kblk)
                    nc.gpsimd.dma_start(
                        fb[:, 0 : hi * T_].rearrange("p (k t) -> p k t", k=hi),
                        feats[:, kf * kblk : kf * kblk + hi, :],
                    )
                ft = fb[:, rf * T_ : (rf + 1) * T_]

                # --- score adds ---
                for i in range(a_rows):
                    nc.scalar.add(
                        sA[:, i * T_ : (i + 1) * T_],
                        tbA[:, i * T_ : (i + 1) * T_],
                        v[:, i : i + 1],
                    )
                vP = v[:, a_rows:T_].unsqueeze(1).broadcast_to([BL_, T_, p_rows])
                nc.gpsimd.tensor_tensor(sP3, tbP3, vP, add)

                # --- max over i ---
                # in-place contiguous tree over sA rows: apad -> 1
                w = apad // 2
                mxA = smpool.tile([BL_, T_], f32, tag="mxA")
                while w >= 1:
                    i0 = sA3[:, 0:w, :]
                    i1 = sA3[:, w : 2 * w, :]
                    out = i0 if w > 1 else mxA[:, :].rearrange(
                        "p (i j) -> p i j", i=1
                    )
                    nc.vector.scalar_tensor_tensor(out, i0, 1.0, i1, mul_op, mx_op)
                    w //= 2
                mxP = smpool.tile([BL_, T_], f32, tag="mxP")
                nc.vector.tensor_reduce(mxP[:, :], sP3, axis=X, op=mx_op)

                # --- combine + feat -> vs slot (the new v) ---
                r = (t - 1) % kblk
                if r == 0:
                    vsb = vspool.tile([BL_, kblk * T_], f32, tag="vsb")
                vslot = vsb[:, r * T_ : (r + 1) * T_]
                nc.vector.scalar_tensor_tensor(
                    mxA[:, :], mxA[:, :], 1.0, mxP[:, :], mul_op, mx_op
                )
                nc.vector.scalar_tensor_tensor(
                    vslot, mxA[:, :], 1.0, ft, mul_op, add
                )
                v = vslot

                if r == kblk - 1 or t == S_ - 1:
                    lo = (t - 1) - r
                    n = r + 1
                    nc.gpsimd.dma_start(
                        vs_out[:, lo : lo + n, :],
                        vsb[:, 0 : n * T_].rearrange("p (k t) -> p k t", k=n),
                    )
    nc.finalize()
    return nc


def build_viterbi_v5_nc(trans_np, S_=S, T_=T, BL_=BL, kblk=8, a_rows=32,
                        d_rows=16):
    """Pipelined 3-engine fp32 kernel (exact arithmetic).

    Row split of the score add s[b,i,j] = t[i,j] + v[b,i]:
      - Act rows [0, a): per-row activation adds into sA ([i,j] layout).
      - DVE rows [a, a+d) and Pool rows [a+d, T): both write one shared
        compact transposed buffer sDP[b, j, k] (k = i - a), so ONE
        contiguous tensor_reduce covers both regions.
    DVE owns all maxes: in-place tree over sA (a must be a power of two),
    contiguous reduce over sDP in two j-halves, combine + feat per half.
    vn half 0 (j < T/2) is emitted first so Act's next-step rows (i < a <=
    T/2) and DVE's own adds can start while the second half is still being
    reduced — that cross-step overlap is what keeps Act/Pool busy during
    DVE's reduce phase.
    """
    import concourse.bacc as bacc
    import concourse.mybir as mybir
    import concourse.tile as tile

    f32 = mybir.dt.float32
    add = mybir.AluOpType.add
    mx_op = mybir.AluOpType.max
    mul_op = mybir.AluOpType.mult
    X = mybir.AxisListType.X

    p_rows = T_ - a_rows - d_rows
    dp = d_rows + p_rows
    H = T_ // 2
    assert (a_rows & (a_rows - 1)) == 0 and a_rows <= H

    nc = bacc.Bacc("TRN2", target_bir_lowering=False, debug=False)
    feats = nc.declare_dram_parameter("feats", [BL_, S_, T_], f32, isOutput=False)
    vs_out = nc.declare_dram_parameter("vs", [BL_, S_ - 1, T_], f32, isOutput=True)

    tblA = np.ascontiguousarray(trans_np[0:a_rows, :].reshape(1, a_rows * T_),
                                dtype=np.float32)
    tblA_d = nc.inline_tensor(tblA, "tblA")
    # shared compact transposed table: tDP[j, k] = trans[a + k, j]
    tblDP = np.ascontiguousarray(trans_np[a_rows:, :].T.reshape(1, T_ * dp),
                                 dtype=np.float32)
    tblDP_d = nc.inline_tensor(tblDP, "tblDP")

    with tile.TileContext(nc) as tc:
        with (
            tc.tile_pool(name="const", bufs=1) as cpool,
            tc.tile_pool(name="feat", bufs=2) as fpool,
            tc.tile_pool(name="vsb", bufs=2) as vspool,
            tc.tile_pool(name="sc", bufs=1) as scpool,
            tc.tile_pool(name="small", bufs=2) as smpool,
        ):
            tbA = cpool.tile([BL_, a_rows * T_], f32, tag="tbA")
            nc.gpsimd.dma_start(tbA[:, :], tblA_d[:, :].partition_broadcast(BL_))
            tbDP = cpool.tile([BL_, T_ * dp], f32, tag="tbDP")
            nc.gpsimd.dma_start(tbDP[:, :], tblDP_d[:, :].partition_broadcast(BL_))
            tbDP3 = tbDP[:, :].rearrange("p (j k) -> p j k", j=T_)

            sA = scpool.tile([BL_, a_rows * T_], f32, tag="sA")
            sA3 = sA[:, :].rearrange("p (i j) -> p i j", i=a_rows)
            sDP = scpool.tile([BL_, T_ * dp], f32, tag="sDP")
            sDP3 = sDP[:, :].rearrange("p (j k) -> p j k", j=T_)

            fb = fpool.tile([BL_, kblk * T_], f32, tag="fb")
            nc.gpsimd.dma_start(
                fb[:, :].rearrange("p (k t) -> p k t", k=kblk), feats[:, 0:kblk, :]
            )
            v = fb[:, 0:T_]  # v_0 = feats[:,0] (start folded by host)

            vsb = vspool.tile([BL_, kblk * T_], f32, tag="vsb")

            D0, D1 = a_rows, a_rows + d_rows
            for t in range(1, S_):
                kf, rf = divmod(t, kblk)
                if rf == 0:
                    fb = fpool.tile([BL_, kblk * T_], f32, tag="fb")
                    hi = min(kblk, S_ - kf * kblk)
                    nc.gpsimd.dma_start(
                        fb[:, 0 : hi * T_].rearrange("p (k t) -> p k t", k=hi),
                        feats[:, kf * kblk : kf * kblk + hi, :],
                    )
                ft = fb[:, rf * T_ : (rf + 1) * T_]

                # --- score adds (Act needs only vn half 0 of step t-1) ---
                for i in range(a_rows):
                    nc.scalar.add(
                        sA[:, i * T_ : (i + 1) * T_],
                        tbA[:, i * T_ : (i + 1) * T_],
                        v[:, i : i + 1],
                    )
                if d_rows:
                    vD = v[:, D0:D1].unsqueeze(1).broadcast_to([BL_, T_, d_rows])
                    nc.vector.scalar_tensor_tensor(
                        sDP3[:, :, 0:d_rows], tbDP3[:, :, 0:d_rows], 1.0, vD,
                        mul_op, add,
                    )
                vP = v[:, D1:T_].unsqueeze(1).broadcast_to([BL_, T_, p_rows])
                nc.gpsimd.tensor_tensor(
                    sDP3[:, :, d_rows:dp], tbDP3[:, :, d_rows:dp], vP, add
                )

                # --- maxes on DVE ---
                mxA = smpool.tile([BL_, T_], f32, tag="mxA")
                w = a_rows // 2
                while w >= 1:
                    i0 = sA3[:, 0:w, :]
                    i1 = sA3[:, w : 2 * w, :]
                    out = i0 if w > 1 else mxA[:, :].rearrange(
                        "p (i j) -> p i j", i=1
                    )
                    nc.vector.scalar_tensor_tensor(out, i0, 1.0, i1, mul_op, mx_op)
                    w //= 2

                r = (t - 1) % kblk
                if r == 0:
                    vsb = vspool.tile([BL_, kblk * T_], f32, tag="vsb")
                vslot = vsb[:, r * T_ : (r + 1) * T_]
                mxP = smpool.tile([BL_, T_], f32, tag="mxP")
                for h0, h1 in ((0, H), (H, T_)):
                    nc.vector.tensor_reduce(
                        mxP[:, h0:h1], sDP3[:, h0:h1, :], axis=X, op=mx_op
                    )
                    nc.vector.scalar_tensor_tensor(
                        mxA[:, h0:h1], mxA[:, h0:h1], 1.0, mxP[:, h0:h1],
                        mul_op, mx_op,
                    )
                    nc.vector.scalar_tensor_tensor(
                        vslot[:, h0:h1], mxA[:, h0:h1], 1.0, ft[:, h0:h1],
                        mul_op, add,
                    )
                v = vslot

                if r == kblk - 1 or t == S_ - 1:
                    lo = (t - 1) - r
                    n = r + 1
                    nc.gpsimd.dma_start(
                        vs_out[:, lo : lo + n, :],
                        vsb[:, 0 : n * T_].rearrange("p (k t) -> p k t", k=n),
                    )
    nc.finalize()
    return nc


def build_viterbi_v6_nc(trans_np, S_=S, T_=T, BL_=BL, kblk=8, jp=72,
                        pchunk=2):
    """j-split DVE/Pool kernel, fp32 exact, all-contiguous [j, i] layout.

    Per step, columns j of the score matrix s[b,j,i] = v[b,i] + tT[j,i] are
    split: DVE computes js = [0, T-jp) with one STT add, Pool computes
    [T-jp, T) in `pchunk` contiguous chunks into its own buffer. DVE owns
    every max: it reduces its own slice while Pool streams, then reduces
    Pool's chunks as they land, then vn = mx + feat. Pool's adds for step
    t+1 overlap DVE's reduce phase of step t only up to the vn dependency,
    so the period is max(DVE busy, Pool chain + last chunk reduce + vn).
    All reduces are contiguous (1.051 ns/elem) and every instruction waits
    on at most one foreign semaphore.
    """
    import concourse.bacc as bacc
    import concourse.mybir as mybir
    import concourse.tile as tile

    f32 = mybir.dt.float32
    add = mybir.AluOpType.add
    mx_op = mybir.AluOpType.max
    mul_op = mybir.AluOpType.mult
    X = mybir.AxisListType.X

    jd = T_ - jp
    # uneven chunks: equal big chunks + a smaller last chunk to shrink the
    # post-Pool tail (last-chunk reduce + vn sit on the critical chain)
    last = max(8, jp // (2 * pchunk))
    big = (jp - last) // (pchunk - 1) if pchunk > 1 else 0
    chunks = [big] * (pchunk - 1) + [jp - big * (pchunk - 1)] if pchunk > 1 else [jp]
    assert sum(chunks) == jp

    nc = bacc.Bacc("TRN2", target_bir_lowering=False, debug=False)
    feats = nc.declare_dram_parameter("feats", [BL_, S_, T_], f32, isOutput=False)
    vs_out = nc.declare_dram_parameter("vs", [BL_, S_ - 1, T_], f32, isOutput=True)

    tT = np.ascontiguousarray(trans_np.T, dtype=np.float32)  # [j, i]
    tD = np.ascontiguousarray(tT[0:jd].reshape(1, jd * T_))
    tP = np.ascontiguousarray(tT[jd:].reshape(1, jp * T_))
    tD_d = nc.inline_tensor(tD, "tD")
    tP_d = nc.inline_tensor(tP, "tP")

    with tile.TileContext(nc) as tc:
        with (
            tc.tile_pool(name="const", bufs=1) as cpool,
            tc.tile_pool(name="feat", bufs=2) as fpool,
            tc.tile_pool(name="vsb", bufs=2) as vspool,
            tc.tile_pool(name="sc", bufs=1) as scpool,
            tc.tile_pool(name="small", bufs=2) as smpool,
        ):
            tbD = cpool.tile([BL_, jd * T_], f32, tag="tbD")
            nc.gpsimd.dma_start(tbD[:, :], tD_d[:, :].partition_broadcast(BL_))
            tbD3 = tbD[:, :].rearrange("p (j i) -> p j i", j=jd)
            tbP = cpool.tile([BL_, jp * T_], f32, tag="tbP")
            nc.gpsimd.dma_start(tbP[:, :], tP_d[:, :].partition_broadcast(BL_))

            sD = scpool.tile([BL_, jd * T_], f32, tag="sD")
            sD3 = sD[:, :].rearrange("p (j i) -> p j i", j=jd)
            sP = scpool.tile([BL_, jp * T_], f32, tag="sP")

            fb = fpool.tile([BL_, kblk * T_], f32, tag="fb")
            nc.gpsimd.dma_start(
                fb[:, :].rearrange("p (k t) -> p k t", k=kblk), feats[:, 0:kblk, :]
            )
            v = fb[:, 0:T_]  # v_0 = feats[:,0] (start folded by host)

            vsb = vspool.tile([BL_, kblk * T_], f32, tag="vsb")

            for t in range(1, S_):
                kf, rf = divmod(t, kblk)
                if rf == 0:
                    fb = fpool.tile([BL_, kblk * T_], f32, tag="fb")
                    hi = min(kblk, S_ - kf * kblk)
                    nc.scalar.dma_start(
                        fb[:, 0 : hi * T_].rearrange("p (k t) -> p k t", k=hi),
                        feats[:, kf * kblk : kf * kblk + hi, :],
                    )
                ft = fb[:, rf * T_ : (rf + 1) * T_]

                mxt = smpool.tile([BL_, T_], f32, tag="mxt")

                # Pool: its j-slice in contiguous chunks (own buffer)
                off = 0
                for c, w in enumerate(chunks):
                    lo, hi_ = off, off + w
                    off = hi_
                    jc = w
                    vC = v[:, :].unsqueeze(1).broadcast_to([BL_, jc, T_])
                    nc.gpsimd.tensor_tensor(
                        sP[:, lo * T_ : hi_ * T_].rearrange(
                            "p (j i) -> p j i", j=jc
                        ),
                        tbP[:, lo * T_ : hi_ * T_].rearrange(
                            "p (j i) -> p j i", j=jc
                        ),
                        vC,
                        add,
                    )

                # DVE: own slice add + reduce, then Pool-chunk reduces
                vD = v[:, :].unsqueeze(1).broadcast_to([BL_, jd, T_])
                nc.vector.scalar_tensor_tensor(sD3, vD, 1.0, tbD3, mul_op, add)
                nc.vector.tensor_reduce(mxt[:, 0:jd], sD3, axis=X, op=mx_op)
                off = 0
                for c, w in enumerate(chunks):
                    lo, hi_ = off, off + w
                    off = hi_
                    nc.vector.tensor_reduce(
                        mxt[:, jd + lo : jd + hi_],
                        sP[:, lo * T_ : hi_ * T_].rearrange(
                            "p (j i) -> p j i", j=w
                        ),
                        axis=X,
                        op=mx_op,
                    )

                r = (t - 1) % kblk
                if r == 0:
                    vsb = vspool.tile([BL_, kblk * T_], f32, tag="vsb")
                vslot = vsb[:, r * T_ : (r + 1) * T_]
                nc.vector.scalar_tensor_tensor(
                    vslot, mxt[:, :], 1.0, ft, mul_op, add
                )
                v = vslot

                if r == kblk - 1 or t == S_ - 1:
                    lo = (t - 1) - r
                    n = r + 1
                    nc.scalar.dma_start(
                        vs_out[:, lo : lo + n, :],
                        vsb[:, 0 : n * T_].rearrange("p (k t) -> p k t", k=n),
                    )
    nc.finalize()
    return nc


def build_viterbi_v7_nc(trans_np, S_=S, T_=T, BL_=BL, kblk=4, jp=86,
                        pchunk=4):
    """Bidirectional j-split kernel: forward chain (t = 1..tau) and backward
    chain (t = S-2..tau) interleaved, tau = S//2. The chains are data-
    independent, so Pool computes one chain's score adds while DVE reduces
    the other's — removing the add/reduce alternation stall of the
    unidirectional kernels.

    fwd:  v_t[j] = max_i(v[i] + tT[j,i]) + feat_t[j]         ([j,i] table)
    bwd:  h = feat'_{t+1} + G_{t+1};  G_t[i] = max_j(h[j] + t[i,j])
          ([i,j] table; feat'[S-1] has stop folded, G_{S-1} = 0)
    Host decodes [0..tau] from the v stream, picks tag_tau =
    argmax(v_tau + G_tau), and forward-traces [tau..S-1] from the G stream.

    Tables are stored fp16 (halves SBUF so both chains fit); scores are
    fp32 with a single rounding, so only the table quantization perturbs
    results (measured harmless). Pool writes its j-chunks into bufs=2
    chunk tiles; DVE reduces each chunk as it lands.
    """
    import concourse.bacc as bacc
    import concourse.mybir as mybir
    import concourse.tile as tile

    f32 = mybir.dt.float32
    f16 = mybir.dt.float16
    add = mybir.AluOpType.add
    mx_op = mybir.AluOpType.max
    mul_op = mybir.AluOpType.mult
    X = mybir.AxisListType.X

    jd = T_ - jp
    tau = S_ // 2
    nF = tau          # fwd steps t = 1..tau, stream rows 0..nF-1
    nB = S_ - 1 - tau  # bwd steps t = S-2..tau, stream rows 0..nB-1

    # chunk widths: equal-ish with a smaller last chunk
    last = max(8, jp // (2 * pchunk))
    big = (jp - last) // (pchunk - 1) if pchunk > 1 else 0
    chunks = [big] * (pchunk - 1) + [jp - big * (pchunk - 1)] if pchunk > 1 else [jp]

    nc = bacc.Bacc("TRN2", target_bir_lowering=False, debug=False)
    feats = nc.declare_dram_parameter("feats", [BL_, S_, T_], f32, isOutput=False)
    vsF = nc.declare_dram_parameter("vsF", [BL_, nF, T_], f32, isOutput=True)
    vsB = nc.declare_dram_parameter("vsB", [BL_, nB, T_], f32, isOutput=True)

    t16 = trans_np.astype(np.float16)
    tTf = np.ascontiguousarray(t16.T.reshape(1, T_ * T_))  # [j, i] for fwd
    tPf = np.ascontiguousarray(t16.reshape(1, T_ * T_))    # [i, j] for bwd
    tT_d = nc.inline_tensor(tTf, "tTf")
    tP_d = nc.inline_tensor(tPf, "tPf")

    with tile.TileContext(nc) as tc:
        with (
            tc.tile_pool(name="const", bufs=1) as cpool,
            tc.tile_pool(name="featF", bufs=2) as fFpool,
            tc.tile_pool(name="featB", bufs=2) as fBpool,
            tc.tile_pool(name="vsbF", bufs=2) as vFpool,
            tc.tile_pool(name="vsbB", bufs=2) as vBpool,
            tc.tile_pool(name="scD", bufs=1) as sdpool,
            tc.tile_pool(name="scPF", bufs=2) as spFpool,
            tc.tile_pool(name="scPB", bufs=2) as spBpool,
            tc.tile_pool(name="small", bufs=3) as smpool,
        ):
            tbF = cpool.tile([BL_, T_ * T_], f16, tag="tbF")
            nc.gpsimd.dma_start(tbF[:, :], tT_d[:, :].partition_broadcast(BL_))
            tbB = cpool.tile([BL_, T_ * T_], f16, tag="tbB")
            nc.gpsimd.dma_start(tbB[:, :], tP_d[:, :].partition_broadcast(BL_))

            sDF = sdpool.tile([BL_, jd * T_], f32, tag="sDF")
            sDF3 = sDF[:, :].rearrange("p (j i) -> p j i", j=jd)
            sDB = sdpool.tile([BL_, jd * T_], f32, tag="sDB")
            sDB3 = sDB[:, :].rearrange("p (j i) -> p j i", j=jd)

            fbF = fFpool.tile([BL_, kblk * T_], f32, tag="fbF")
            nc.scalar.dma_start(
                fbF[:, :].rearrange("p (k t) -> p k t", k=kblk),
                feats[:, 0:kblk, :],
            )
            qb0 = S_ - kblk
            fbB = fBpool.tile([BL_, kblk * T_], f32, tag="fbB")
            nc.scalar.dma_start(
                fbB[:, :].rearrange("p (k t) -> p k t", k=kblk),
                feats[:, qb0:S_, :],
            )

            vF = fbF[:, 0:T_]   # v_0 (start folded by host)
            hB = fbB[:, (S_ - 1 - qb0) * T_ : (S_ - qb0) * T_]  # feat'_{S-1}

            vsFb = vFpool.tile([BL_, kblk * T_], f32, tag="vsFb")
            vsBb = vBpool.tile([BL_, kblk * T_], f32, tag="vsBb")

            def chain_step(tbl, sD3, vsrc, mxt):
                """one j-split step: helpers+DVE adds, DVE reduces into mxt"""
                tb3d = tbl[:, 0 : jd * T_].rearrange("p (j i) -> p j i", j=jd)
                vD = vsrc.unsqueeze(1).broadcast_to([BL_, jd, T_])
                nc.vector.scalar_tensor_tensor(sD3, tb3d, 1.0, vD, mul_op, add)
                nc.vector.tensor_reduce(mxt[:, 0:jd], sD3, axis=X, op=mx_op)

            for k in range(nF):
                # ---------- forward step t = 1 + k ----------
                t = 1 + k
                kf, rf = divmod(t, kblk)
                if rf == 0:
                    fbF = fFpool.tile([BL_, kblk * T_], f32, tag="fbF")
                    hi = min(kblk, S_ - kf * kblk)
                    nc.scalar.dma_start(
                        fbF[:, 0 : hi * T_].rearrange("p (k t) -> p k t", k=hi),
                        feats[:, kf * kblk : kf * kblk + hi, :],
                    )
                ftF = fbF[:, rf * T_ : (rf + 1) * T_]

                mxF = smpool.tile([BL_, T_], f32, tag="mxF")
                # Pool chunks for fwd
                pf_tiles = []
                off = jd
                for w in chunks:
                    sPF = spFpool.tile([BL_, w * T_], f32, tag="sPF")
                    vC = vF[:, :].unsqueeze(1).broadcast_to([BL_, w, T_])
                    nc.gpsimd.tensor_tensor(
                        sPF[:, :].rearrange("p (j i) -> p j i", j=w),
                        tbF[:, off * T_ : (off + w) * T_].rearrange(
                            "p (j i) -> p j i", j=w
                        ),
                        vC,
                        add,
                    )
                    pf_tiles.append((sPF, off, w))
                    off += w
                chain_step(tbF, sDF3, vF[:, :], mxF)
                for sPF, off_, w in pf_tiles:
                    nc.vector.tensor_reduce(
                        mxF[:, off_ : off_ + w],
                        sPF[:, :].rearrange("p (j i) -> p j i", j=w),
                        axis=X,
                        op=mx_op,
                    )
                r = t - 1  # stream row
                if r % kblk == 0:
                    vsFb = vFpool.tile([BL_, kblk * T_], f32, tag="vsFb")
                vslotF = vsFb[:, (r % kblk) * T_ : (r % kblk + 1) * T_]
                nc.vector.scalar_tensor_tensor(
                    vslotF, mxF[:, :], 1.0, ftF, mul_op, add
                )
                vF = vslotF
                if r % kblk == kblk - 1 or r == nF - 1:
                    lo = r - (r % kblk)
                    n = (r % kblk) + 1
                    nc.scalar.dma_start(
                        vsF[:, lo : lo + n, :],
                        vsFb[:, 0 : n * T_].rearrange("p (k t) -> p k t", k=n),
                    )

                # ---------- backward step t = S-2-k (if any) ----------
                if k < nB:
                    t_b = S_ - 2 - k
                    q = t_b + 1  # feat'_{t+1} index, descending from S-1
                    mxB = smpool.tile([BL_, T_], f32, tag="mxB")
                    pb_tiles = []
                    off = jd
                    for w in chunks:
                        sPB = spBpool.tile([BL_, w * T_], f32, tag="sPB")
                        hC = hB.unsqueeze(1).broadcast_to([BL_, w, T_])
                        nc.gpsimd.tensor_tensor(
                            sPB[:, :].rearrange("p (j i) -> p j i", j=w),
                            tbB[:, off * T_ : (off + w) * T_].rearrange(
                                "p (j i) -> p j i", j=w
                            ),
                            hC,
                            add,
                        )
                        pb_tiles.append((sPB, off, w))
                        off += w
                    chain_step(tbB, sDB3, hB, mxB)
                    for sPB, off_, w in pb_tiles:
                        nc.vector.tensor_reduce(
                            mxB[:, off_ : off_ + w],
                            sPB[:, :].rearrange("p (j i) -> p j i", j=w),
                            axis=X,
                            op=mx_op,
                        )
                    # G_t = mxB; stream row m = t_b - tau descending
                    m = t_b - tau
                    if m % kblk == kblk - 1 or m == nB - 1:
                        vsBb = vBpool.tile([BL_, kblk * T_], f32, tag="vsBb")
                    gslot = vsBb[:, (m % kblk) * T_ : (m % kblk + 1) * T_]
                    nc.vector.tensor_copy(gslot, mxB[:, :])
                    if m % kblk == 0:
                        n = kblk if (m + kblk <= nB) else (nB - m)
                        # rows [m, m+n) are in the buffer (written descending)
                        nc.scalar.dma_start(
                            vsB[:, m : m + n, :],
                            vsBb[:, 0 : n * T_].rearrange(
                                "p (k t) -> p k t", k=n
                            ),
                        )
                    # next h = feat'_{t_b} + G_{t_b}  (for the NEXT bwd step)
                    if k + 1 < nB:
                        qn = t_b  # next step's feat index
                        if qn % kblk == kblk - 1:
                            fbB = fBpool.tile([BL_, kblk * T_], f32, tag="fbB")
                            lo_q = qn - (kblk - 1)
                            nc.scalar.dma_start(
                                fbB[:, :].rearrange("p (k t) -> p k t", k=kblk),
                                feats[:, lo_q : lo_q + kblk, :],
                            )
                            qblo = lo_q
                        else:
                            qblo = qn - (qn % kblk)
                        hnew = smpool.tile([BL_, T_], f32, tag="hB")
                        nc.vector.scalar_tensor_tensor(
                            hnew[:, :],
                            mxB[:, :],
                            1.0,
                            fbB[:, (qn - qblo) * T_ : (qn - qblo + 1) * T_],
                            mul_op,
                            add,
                        )
                        hB = hnew[:, :]
    nc.finalize()
    return nc


def _install_ntff_hook_shim():
    """The agent image's `antenv` lacks `axon_hooks`, so trn_boot degrades
    silently and bass_utils' trace path crashes on import. Provide the same
    ctypes-based NTFF hook trn_boot would have registered."""
    import sys
    import types

    if "antenv.axon_hooks" in sys.modules:
        return
    try:
        import antenv.axon_hooks  # noqa: F401
        return
    except ImportError:
        pass
    try:
        from trn_agent_boot.trn_boot import _ntff_profile_via_ctypes

        hook = _ntff_profile_via_ctypes("/opt/axon/libaxon_pjrt.so")
    except Exception:
        hook = None
    m = types.ModuleType("antenv.axon_hooks")
    m._hook = hook
    m.get_axon_ntff_profile_hook = lambda: m._hook
    def _set(h):
        m._hook = h
    m.set_axon_ntff_profile_hook = _set
    sys.modules["antenv.axon_hooks"] = m


def _run(nc, in_maps, **kwargs):
    if kwargs.get("trace"):
        _install_ntff_hook_shim()
    from concourse.bass_utils import run_bass_kernel_spmd

    return run_bass_kernel_spmd(nc, in_maps, core_ids=list(range(len(in_maps))), **kwargs)


def _backtrace_from_vs(vs, v0, trans, stop):
    """Exact backtrace from per-step state vectors.

    vs: [B, S-1, T] fp32 (v at t=1..S-1), v0: [B, T] (v at t=0).
    Recomputes argmax_i(v[t-1,:,i] + trans[i, j_t]) along the traced path
    only — identical fp32 arithmetic + first-index ties as the reference.
    """
    B_, Sm1, T_ = vs.shape
    S_ = Sm1 + 1
    last = np.argmax(vs[:, -1, :] + stop[None, :], axis=1).astype(np.int32)
    tags = np.empty((B_, S_), dtype=np.int32)
    tags[:, -1] = last
    cur = last
    transT = np.ascontiguousarray(trans.T)  # [j, i]
    for t in range(S_ - 1, 0, -1):
        vprev = vs[:, t - 2, :] if t >= 2 else v0
        col = vprev + transT[cur]  # [B, T] fp32: v[b,t-1,i] + trans[i, j_t]
        cur = np.argmax(col, axis=1).astype(np.int32)
        tags[:, t - 1] = cur
    return tags


def kernel(feats, transitions, start_transitions, stop_transitions, _trace=False,
           _variant="v6"):
    feats = np.asarray(feats, dtype=np.float32).copy()
    trans = np.ascontiguousarray(np.asarray(transitions, dtype=np.float32))
    start = np.ascontiguousarray(np.asarray(start_transitions, dtype=np.float32))
    stop = np.ascontiguousarray(np.asarray(stop_transitions, dtype=np.float32))
    assert feats.shape == (B, S, T)

    feats[:, 0, :] += start  # fold start_transitions (bit-exact vs reference)

    if _variant == "v7":
        import os as _os
        feats[:, S - 1, :] += stop  # fold stop for the backward chain
        nc = build_viterbi_v7_nc(
            trans,
            jp=int(_os.environ.get("VT_JP", "86")),
            pchunk=int(_os.environ.get("VT_PCHUNK", "4")),
            kblk=int(_os.environ.get("VT_KBLK", "4")),
        )
    elif _variant == "v6":
        import os as _os
        nc = build_viterbi_v6_nc(
            trans,
            jp=int(_os.environ.get("VT_JP", "72")),
            pchunk=int(_os.environ.get("VT_PCHUNK", "3")),
            kblk=int(_os.environ.get("VT_KBLK", "8")),
        )
    elif _variant == "v5":
        import os as _os
        nc = build_viterbi_v5_nc(
            trans,
            a_rows=int(_os.environ.get("VT_AROWS", "32")),
            d_rows=int(_os.environ.get("VT_DROWS", "16")),
            kblk=int(_os.environ.get("VT_KBLK", "8")),
        )
    elif _variant == "v4":
        import os as _os
        nc = build_viterbi_v4_nc(
            trans,
            a_rows=int(_os.environ.get("VT_AROWS", "58")),
            kblk=int(_os.environ.get("VT_KBLK", "8")),
        )
    elif _variant == "f16":
        nc = build_viterbi_f16_nc(trans)
    else:
        nc = build_viterbi_nc(trans, variant=_variant)
    in_maps = [{"feats": feats[c * BL : (c + 1) * BL]} for c in range(NCORES)]
    res = _run(nc, in_maps, trace=_trace)

    if _variant == "v7":
        tau = S // 2
        vsF = np.concatenate([r["vsF"] for r in res.results], axis=0)  # [B, tau, T]
        vsB = np.concatenate([r["vsB"] for r in res.results], axis=0)  # [B, S-1-tau, T]
        t16 = trans.astype(np.float16).astype(np.float32)
        t16T = np.ascontiguousarray(t16.T)
        v0 = feats[:, 0, :]  # start folded
        tags = np.empty((B, S), dtype=np.int32)
        cur = np.argmax(vsF[:, tau - 1] + vsB[:, 0], axis=1).astype(np.int32)
        tags[:, tau] = cur
        # forward segment [0..tau-1]: same backtrace as before, fp16 table
        for t in range(tau, 0, -1):
            vprev = vsF[:, t - 2, :] if t >= 2 else v0
            cur = np.argmax(vprev + t16T[cur], axis=1).astype(np.int32)
            tags[:, t - 1] = cur
        # backward segment [tau+1..S-1]: forward-trace on the G stream
        cur = tags[:, tau].copy()
        for t in range(tau, S - 1):
            q = t + 1
            h = feats[:, q, :].copy()  # stop already folded into feats[S-1]
            if q <= S - 2:
                h = h + vsB[:, q - tau, :]
            cur = np.argmax(t16[cur] + h, axis=1).astype(np.int32)
            tags[:, q] = cur
    elif _variant in ("f16", "v4", "v5", "v6"):
        vs = np.concatenate([r["vs"] for r in res.results], axis=0)  # [B, S-1, T]
        v0 = feats[:, 0, :]  # start already folded
        tags = _backtrace_from_vs(vs, v0, trans, stop)
    elif _variant == "v0":
        bp_f = np.concatenate(
            [np.transpose(r["bp"], (1, 0, 2)) for r in res.results], axis=0
        )
        v_fin = np.concatenate([r["v_final"] for r in res.results], axis=0)
        idx = (T - 1) - bp_f.astype(np.int32)
        last = np.argmax(v_fin + stop[None, :], axis=1).astype(np.int32)
        tags = np.empty((B, S), dtype=np.int32)
        tags[:, S - 1] = last
        cur = last
        ar = np.arange(B)
        for t in range(S - 2, -1, -1):
            cur = idx[ar, t, cur]
            tags[:, t] = cur
    else:
        vs = np.concatenate(
            [np.transpose(r["vs"], (1, 0, 2)) for r in res.results], axis=0
        )  # [B, S-1, T]
        v0 = feats[:, 0, :]  # start already folded
        tags = _backtrace_from_vs(vs, v0, trans, stop)

    if _trace:
        return tags, res
    return tags



# revision 19
# speedup vs baseline: 1.4466x; 1.4466x over previous
"""Viterbi decode (CRF layer) on Trainium2 — Bass kernel.

Problem: feats [1024, 512, 128] f32, transitions [128, 128],
start/stop_transitions [128] -> best tag sequence [1024, 512] int32.

Strategy: pure batch data-parallelism across 8 NeuronCores. Each core takes
128 batch rows (= 128 SBUF partitions) and runs the sequential max-plus
forward scan on-chip:

    sc[b, i, j] = v[b, i] + trans[i, j]          (fp32, one rounding)
    mx[b, j]    = max_i sc[b, i, j]
    v'[b, j]    = mx[b, j] + feats[b, t, j]      (fp32, one rounding)

The per-step state vectors v stream to DRAM; the backtrace recomputes the
argmax only along the traced path (B*S tiny argmaxes) on host during the
unshard step, with identical fp32 arithmetic and first-index tie-breaking,
so the final int32 tags match the reference bit-exactly.

variant="v0" keeps the full device-side backpointer computation (slower,
fully self-contained backpointers) as a fallback.
"""

import numpy as np

B, S, T = 1024, 512, 128
NCORES = 8
BL = B // NCORES  # 128 batch rows per core == SBUF partition count


def build_viterbi_nc(trans_np, S_=S, T_=T, BL_=BL, variant="v1"):
    """Build the per-core Bass program (same NEFF for all cores).

    NOTE: start_transitions must already be folded into feats[:, 0, :] by the
    caller (bit-exact: same single fp32 add the reference performs).

    walrus/core_v3 allows only ONE attached sync-wait per compute
    instruction; the initial state goes through a DVE tensor_copy so every
    instruction waits on at most one foreign semaphore.
    """
    import concourse.bacc as bacc
    import concourse.mybir as mybir
    import concourse.tile as tile

    f32 = mybir.dt.float32
    add = mybir.AluOpType.add
    mx_op = mybir.AluOpType.max
    eq_op = mybir.AluOpType.is_equal
    mul_op = mybir.AluOpType.mult
    X = mybir.AxisListType.X

    nc = bacc.Bacc("TRN2", target_bir_lowering=False, debug=False)
    feats = nc.declare_dram_parameter("feats", [BL_, S_, T_], f32, isOutput=False)
    if variant == "v0":
        bp = nc.declare_dram_parameter("bp", [S_ - 1, BL_, T_], f32, isOutput=True)
    else:
        vs_out = nc.declare_dram_parameter("vs", [S_ - 1, BL_, T_], f32, isOutput=True)
    v_final = nc.declare_dram_parameter("v_final", [BL_, T_], f32, isOutput=True)

    if variant == "v2":
        # table stored [j, i] (transposed) so the score buffer is written and
        # reduced fully contiguously in [b, j, i] order
        tbl = np.ascontiguousarray(trans_np.T.reshape(1, T_ * T_), dtype=np.float32)
    else:
        tbl = np.ascontiguousarray(trans_np.reshape(1, T_ * T_), dtype=np.float32)
    tbc_d = nc.inline_tensor(tbl, "tbc")
    iota_d = nc.inline_tensor(
        np.arange(T_ - 1, -1, -1, dtype=np.float32).reshape(1, T_), "iotad"
    )

    with tile.TileContext(nc) as tc:
        with (
            tc.tile_pool(name="const", bufs=1) as cpool,
            tc.tile_pool(name="feat", bufs=8) as fpool,
            tc.tile_pool(name="vst", bufs=4) as vpool,
            tc.tile_pool(name="sc", bufs=1 if variant == "v0" else 2) as scpool,
            tc.tile_pool(name="mx", bufs=2) as mxpool,
            tc.tile_pool(name="bpp", bufs=4) as bppool,
        ):
            tbc = cpool.tile([BL_, T_ * T_], f32, tag="tbc")
            nc.gpsimd.dma_start(tbc[:, :], tbc_d[:, :].partition_broadcast(BL_))
            iotab = cpool.tile([BL_, T_], f32, tag="iotab")
            nc.gpsimd.dma_start(iotab[:, :], iota_d[:, :].partition_broadcast(BL_))

            f0 = fpool.tile([BL_, T_], f32, tag="feat")
            nc.gpsimd.dma_start(f0[:, :], feats[:, 0, :])
            v = vpool.tile([BL_, T_], f32, tag="v")
            nc.vector.tensor_copy(v[:, :], f0[:, :])

            tb3 = tbc[:, :].rearrange("p (i j) -> p i j", i=T_)
            io3 = iotab[:, :].unsqueeze(-1).broadcast_to([BL_, T_, T_])
            # v2: table is [j, i]-major; split the add by j between DVE and
            # Pool (Pool ~2x slower -> give it the smaller range)
            import os as _os
            JSPLIT = int(_os.environ.get("VT_JSPLIT", T_))
            DSPLIT = int(_os.environ.get("VT_DSPLIT", T_ // 2))

            for t in range(1, S_):
                ft = fpool.tile([BL_, T_], f32, tag="feat")
                nc.gpsimd.dma_start(ft[:, :], feats[:, t, :])

                sc = scpool.tile([BL_, T_ * T_], f32, tag="sc")
                sc3 = sc[:, :].rearrange("p (i j) -> p i j", i=T_)
                scT = sc[:, :].rearrange("p (i j) -> p j i", i=T_)
                mxt = mxpool.tile([BL_, T_], f32, tag="mx")

                if variant == "v2":
                    # sc[b, j, i] = v[b, i] + tT[j, i]; contiguous writes
                    scJ = sc[:, :].rearrange("p (j i) -> p j i", j=T_)
                    tbJ = tbc[:, :].rearrange("p (j i) -> p j i", j=T_)
                    nA = JSPLIT * T_
                    v3a = v[:, :].unsqueeze(1).broadcast_to([BL_, JSPLIT, T_])
                    scA = sc[:, 0:nA].rearrange("p (j i) -> p j i", j=JSPLIT)
                    tbA = tbc[:, 0:nA].rearrange("p (j i) -> p j i", j=JSPLIT)
                    nc.vector.tensor_tensor(scA, v3a, tbA, add)
                    if JSPLIT < T_:
                        v3b = v[:, :].unsqueeze(1).broadcast_to(
                            [BL_, T_ - JSPLIT, T_]
                        )
                        scB = sc[:, nA : T_ * T_].rearrange(
                            "p (j i) -> p j i", j=T_ - JSPLIT
                        )
                        tbB = tbc[:, nA : T_ * T_].rearrange(
                            "p (j i) -> p j i", j=T_ - JSPLIT
                        )
                        nc.gpsimd.tensor_tensor(scB, v3b, tbB, add)
                    nc.vector.tensor_reduce(mxt[:, :], scJ, axis=X, op=mx_op)
                elif variant == "v3":
                    # sc[b,i,j] = t[i,j] + v[b,i]: DVE does rows [0, DSPLIT)
                    # in one tensor_tensor; ACT does rows [DSPLIT, T) as
                    # per-row activation-adds (bias = per-partition scalar)
                    nD = DSPLIT * T_
                    v3a = v[:, 0:DSPLIT].unsqueeze(-1).broadcast_to(
                        [BL_, DSPLIT, T_]
                    )
                    scA = sc[:, 0:nD].rearrange("p (i j) -> p i j", i=DSPLIT)
                    tbA = tbc[:, 0:nD].rearrange("p (i j) -> p i j", i=DSPLIT)
                    nc.vector.tensor_tensor(scA, v3a, tbA, add)
                    for i in range(DSPLIT, T_):
                        nc.scalar.add(
                            sc[:, i * T_ : (i + 1) * T_],
                            tbc[:, i * T_ : (i + 1) * T_],
                            v[:, i : i + 1],
                        )
                    nc.vector.tensor_reduce(mxt[:, :], scT, axis=X, op=mx_op)
                else:
                    v3 = v[:, :].unsqueeze(-1).broadcast_to([BL_, T_, T_])
                    nc.vector.tensor_tensor(sc3, v3, tb3, add)
                    nc.vector.tensor_reduce(mxt[:, :], scT, axis=X, op=mx_op)

                vn = vpool.tile([BL_, T_], f32, tag="v")
                nc.vector.tensor_tensor(vn[:, :], mxt[:, :], ft[:, :], add)

                if variant == "v0":
                    # backpointers on device: sc <- (sc==mx)*(T-1-i); bp=max_i
                    mx3 = mxt[:, :].unsqueeze(1).broadcast_to([BL_, T_, T_])
                    nc.vector.tensor_tensor(sc3, sc3, mx3, eq_op)
                    nc.vector.tensor_tensor(sc3, sc3, io3, mul_op)
                    bpt = bppool.tile([BL_, T_], f32, tag="bp")
                    nc.vector.tensor_reduce(bpt[:, :], scT, axis=X, op=mx_op)
                    nc.gpsimd.dma_start(bp[t - 1, :, :], bpt[:, :])
                else:
                    nc.gpsimd.dma_start(vs_out[t - 1, :, :], vn[:, :])

                v = vn

            nc.gpsimd.dma_start(v_final[:, :], v[:, :])
    nc.finalize()
    return nc


def build_viterbi_f16_nc(trans_np, S_=S, T_=T, BL_=BL, kblk=8):
    """fp16 forward-scan kernel: per step, the [T,T] score add and the
    max-tree run in fp16 on DVE (4x perf mode); the state update
    vn = max + feat stays fp32, with per-step recentring (subtract the
    per-row max) so fp16 magnitudes stay ~|8|. The recentred fp32 state
    trajectory streams to DRAM b-major; the host backtraces from it in fp32.

    Numerics validated against reference in numpy sim: ~1e-4 tag mismatch
    rate (rel err ~7e-3, gate is 2e-2).
    """
    import concourse.bacc as bacc
    import concourse.mybir as mybir
    import concourse.tile as tile

    f32 = mybir.dt.float32
    f16 = mybir.dt.float16
    add = mybir.AluOpType.add
    mx_op = mybir.AluOpType.max
    mul_op = mybir.AluOpType.mult
    sub_op = mybir.AluOpType.subtract
    X = mybir.AxisListType.X

    nc = bacc.Bacc("TRN2", target_bir_lowering=False, debug=False)
    feats = nc.declare_dram_parameter("feats", [BL_, S_, T_], f32, isOutput=False)
    vs_out = nc.declare_dram_parameter("vs", [BL_, S_ - 1, T_], f32, isOutput=True)

    # table stored [j, i] (transposed) so score writes and the i-tree are
    # contiguous per j
    tbl16 = np.ascontiguousarray(trans_np.T.reshape(1, T_ * T_)).astype(np.float16)
    tbc_d = nc.inline_tensor(tbl16, "tbc16")

    nblk = (S_ + kblk - 1) // kblk  # feat blocks cover s in [0, S)

    with tile.TileContext(nc) as tc:
        with (
            tc.tile_pool(name="const", bufs=1) as cpool,
            tc.tile_pool(name="feat", bufs=2) as fpool,
            tc.tile_pool(name="vsb", bufs=2) as vspool,
            tc.tile_pool(name="sc", bufs=1) as scpool,
            tc.tile_pool(name="small", bufs=2) as smpool,
        ):
            tbc = cpool.tile([BL_, T_ * T_], f16, tag="tbc")
            nc.gpsimd.dma_start(tbc[:, :], tbc_d[:, :].partition_broadcast(BL_))
            t3 = tbc[:, :].rearrange("p (j i) -> p j i", j=T_)

            s16 = scpool.tile([BL_, T_ * T_], f16, tag="s16")
            s3 = s16[:, :].rearrange("p (j i) -> p j i", j=T_)

            # feat block 0 (s = 0..kblk-1)
            fb = fpool.tile([BL_, kblk * T_], f32, tag="fb")
            nc.gpsimd.dma_start(
                fb[:, :].rearrange("p (k t) -> p k t", k=kblk), feats[:, 0:kblk, :]
            )

            # initial state from f0 (host already folded start_transitions)
            f0 = fb[:, 0:T_]
            shift = smpool.tile([BL_, 1], f32, tag="shift")
            nc.vector.tensor_reduce(shift[:, :], f0, axis=X, op=mx_op)
            v16 = smpool.tile([BL_, T_], f16, tag="v16")
            nc.vector.tensor_scalar(v16[:, :], f0, shift[:, :], None, sub_op)

            vsb = vspool.tile([BL_, kblk * T_], f32, tag="vsb")

            for t in range(1, S_):
                kf, rf = divmod(t, kblk)
                if rf == 0:  # need next feat block (covers s = t..t+kblk-1)
                    fb = fpool.tile([BL_, kblk * T_], f32, tag="fb")
                    hi = min(kblk, S_ - kf * kblk)
                    nc.gpsimd.dma_start(
                        fb[:, 0 : hi * T_].rearrange("p (k t) -> p k t", k=hi),
                        feats[:, kf * kblk : kf * kblk + hi, :],
                    )
                ft = fb[:, rf * T_ : (rf + 1) * T_]

                # s16[b,j,i] = fp16(v16[b,i] + t16[j,i])   (4x DVE mode)
                v3 = v16[:, :].unsqueeze(1).broadcast_to([BL_, T_, T_])
                nc.vector.scalar_tensor_tensor(s3, v3, 1.0, t3, mul_op, add)

                # in-place max tree over i: 128 -> 1
                w = T_ // 2
                while w >= 1:
                    a = s3[:, :, 0:w]
                    b = s3[:, :, w : 2 * w]
                    if w > 1:
                        nc.vector.scalar_tensor_tensor(a, a, 1.0, b, mul_op, mx_op)
                    else:
                        mx16 = smpool.tile([BL_, T_], f16, tag="mx16")
                        m3 = mx16[:, :].rearrange("p (j i) -> p j i", j=T_, i=1)
                        nc.vector.scalar_tensor_tensor(m3, a, 1.0, b, mul_op, mx_op)
                    w //= 2

                # vn32 = fp32(mx16) + ft   -> written into the vs block slot
                r = (t - 1) % kblk
                if r == 0:
                    vsb = vspool.tile([BL_, kblk * T_], f32, tag="vsb")
                vslot = vsb[:, r * T_ : (r + 1) * T_]
                nc.vector.scalar_tensor_tensor(vslot, mx16[:, :], 1.0, ft, mul_op, add)

                # recentre: shift = max_j vn; v16 = fp16(vn - shift)
                shift = smpool.tile([BL_, 1], f32, tag="shift")
                nc.vector.tensor_reduce(shift[:, :], vslot, axis=X, op=mx_op)
                v16 = smpool.tile([BL_, T_], f16, tag="v16")
                nc.vector.tensor_scalar(v16[:, :], vslot, shift[:, :], None, sub_op)

                if r == kblk - 1 or t == S_ - 1:  # flush vs block
                    lo = (t - 1) - r  # first vs row in this block
                    n = r + 1
                    nc.gpsimd.dma_start(
                        vs_out[:, lo : lo + n, :],
                        vsb[:, 0 : n * T_].rearrange("p (k t) -> p k t", k=n),
                    )
    nc.finalize()
    return nc


def build_viterbi_v4_nc(trans_np, S_=S, T_=T, BL_=BL, kblk=8, a_rows=58,
                        apad=64):
    """3-engine fp32 kernel. Exact arithmetic (same single-rounding adds as
    the reference), so tags match bit-exactly.

    Score add s[b,i,j] = t[i,j] + v[b,i] split by i-rows:
      - Act: rows [0, a)   -> s_A buffer, [i, j] layout (row-contig), one
        activation-add per row (bias = v[:, i]). Rows [a, apad) are -1e38
        pad written once so DVE can run a fixed power-of-2 max tree.
      - Pool: rows [a, T)  -> s_P buffer in compact TRANSPOSED [j, k] layout
        (one tensor_tensor add; Pool has no max op, DVE reduces contiguous).
    DVE: in-place contiguous max tree over s_A rows (apad -> 1), one
    contiguous tensor_reduce over s_P, combine, + feat -> vs block slot.
    State trajectory streams b-major; host does the exact fp32 backtrace.
    """
    import concourse.bacc as bacc
    import concourse.mybir as mybir
    import concourse.tile as tile

    f32 = mybir.dt.float32
    add = mybir.AluOpType.add
    mx_op = mybir.AluOpType.max
    mul_op = mybir.AluOpType.mult
    X = mybir.AxisListType.X

    p_rows = T_ - a_rows
    assert a_rows <= apad and (apad & (apad - 1)) == 0

    nc = bacc.Bacc("TRN2", target_bir_lowering=False, debug=False)
    feats = nc.declare_dram_parameter("feats", [BL_, S_, T_], f32, isOutput=False)
    vs_out = nc.declare_dram_parameter("vs", [BL_, S_ - 1, T_], f32, isOutput=True)

    # Act table: rows [0, a) of trans, [i, j] layout
    tblA = np.ascontiguousarray(trans_np[0:a_rows, :].reshape(1, a_rows * T_),
                                dtype=np.float32)
    tblA_d = nc.inline_tensor(tblA, "tblA")
    # Pool table: rows [a, T) transposed-compact: tP[j, k] = trans[a+k, j]
    tblP = np.ascontiguousarray(trans_np[a_rows:, :].T.reshape(1, T_ * p_rows),
                                dtype=np.float32)
    tblP_d = nc.inline_tensor(tblP, "tblP")

    with tile.TileContext(nc) as tc:
        with (
            tc.tile_pool(name="const", bufs=1) as cpool,
            tc.tile_pool(name="feat", bufs=2) as fpool,
            tc.tile_pool(name="vsb", bufs=2) as vspool,
            tc.tile_pool(name="sc", bufs=1) as scpool,
            tc.tile_pool(name="small", bufs=2) as smpool,
        ):
            tbA = cpool.tile([BL_, a_rows * T_], f32, tag="tbA")
            nc.gpsimd.dma_start(tbA[:, :], tblA_d[:, :].partition_broadcast(BL_))
            tbP = cpool.tile([BL_, T_ * p_rows], f32, tag="tbP")
            nc.gpsimd.dma_start(tbP[:, :], tblP_d[:, :].partition_broadcast(BL_))
            tbP3 = tbP[:, :].rearrange("p (j k) -> p j k", j=T_)

            sA = scpool.tile([BL_, apad * T_], f32, tag="sA")
            sA3 = sA[:, :].rearrange("p (i j) -> p i j", i=apad)
            if a_rows < apad:  # one-time -inf pad rows for the fixed tree
                nc.vector.memset(sA[:, a_rows * T_ :], -1.0e38)
            sP = scpool.tile([BL_, T_ * p_rows], f32, tag="sP")
            sP3 = sP[:, :].rearrange("p (j k) -> p j k", j=T_)

            fb = fpool.tile([BL_, kblk * T_], f32, tag="fb")
            nc.gpsimd.dma_start(
                fb[:, :].rearrange("p (k t) -> p k t", k=kblk), feats[:, 0:kblk, :]
            )
            v = fb[:, 0:T_]  # v_0 = feats[:,0] (start folded by host)

            vsb = vspool.tile([BL_, kblk * T_], f32, tag="vsb")

            for t in range(1, S_):
                kf, rf = divmod(t, kblk)
                if rf == 0:
                    fb = fpool.tile([BL_, kblk * T_], f32, tag="fb")
                    hi = min(kblk, S_ - kf * kblk)
                    nc.gpsimd.dma_start(
                        fb[:, 0 : hi * T_].rearrange("p (k t) -> p k t", k=hi),
                        feats[:, kf * kblk : kf * kblk + hi, :],
                    )
                ft = fb[:, rf * T_ : (rf + 1) * T_]

                # --- score adds ---
                for i in range(a_rows):
                    nc.scalar.add(
                        sA[:, i * T_ : (i + 1) * T_],
                        tbA[:, i * T_ : (i + 1) * T_],
                        v[:, i : i + 1],
                    )
                vP = v[:, a_rows:T_].unsqueeze(1).broadcast_to([BL_, T_, p_rows])
                nc.gpsimd.tensor_tensor(sP3, tbP3, vP, add)

                # --- max over i ---
                # in-place contiguous tree over sA rows: apad -> 1
                w = apad // 2
                mxA = smpool.tile([BL_, T_], f32, tag="mxA")
                while w >= 1:
                    i0 = sA3[:, 0:w, :]
                    i1 = sA3[:, w : 2 * w, :]
                    out = i0 if w > 1 else mxA[:, :].rearrange(
                        "p (i j) -> p i j", i=1
                    )
                    nc.vector.scalar_tensor_tensor(out, i0, 1.0, i1, mul_op, mx_op)
                    w //= 2
                mxP = smpool.tile([BL_, T_], f32, tag="mxP")
                nc.vector.tensor_reduce(mxP[:, :], sP3, axis=X, op=mx_op)

                # --- combine + feat -> vs slot (the new v) ---
                r = (t - 1) % kblk
                if r == 0:
                    vsb = vspool.tile([BL_, kblk * T_], f32, tag="vsb")
                vslot = vsb[:, r * T_ : (r + 1) * T_]
                nc.vector.scalar_tensor_tensor(
                    mxA[:, :], mxA[:, :], 1.0, mxP[:, :], mul_op, mx_op
                )
                nc.vector.scalar_tensor_tensor(
                    vslot, mxA[:, :], 1.0, ft, mul_op, add
                )
                v = vslot

                if r == kblk - 1 or t == S_ - 1:
                    lo = (t - 1) - r
                    n = r + 1
                    nc.gpsimd.dma_start(
                        vs_out[:, lo : lo + n, :],
                        vsb[:, 0 : n * T_].rearrange("p (k t) -> p k t", k=n),
                    )
    nc.finalize()
    return nc


def build_viterbi_v5_nc(trans_np, S_=S, T_=T, BL_=BL, kblk=8, a_rows=32,
                        d_rows=16):
    """Pipelined 3-engine fp32 kernel (exact arithmetic).

    Row split of the score add s[b,i,j] = t[i,j] + v[b,i]:
      - Act rows [0, a): per-row activation adds into sA ([i,j] layout).
      - DVE rows [a, a+d) and Pool rows [a+d, T): both write one shared
        compact transposed buffer sDP[b, j, k] (k = i - a), so ONE
        contiguous tensor_reduce covers both regions.
    DVE owns all maxes: in-place tree over sA (a must be a power of two),
    contiguous reduce over sDP in two j-halves, combine + feat per half.
    vn half 0 (j < T/2) is emitted first so Act's next-step rows (i < a <=
    T/2) and DVE's own adds can start while the second half is still being
    reduced — that cross-step overlap is what keeps Act/Pool busy during
    DVE's reduce phase.
    """
    import concourse.bacc as bacc
    import concourse.mybir as mybir
    import concourse.tile as tile

    f32 = mybir.dt.float32
    add = mybir.AluOpType.add
    mx_op = mybir.AluOpType.max
    mul_op = mybir.AluOpType.mult
    X = mybir.AxisListType.X

    p_rows = T_ - a_rows - d_rows
    dp = d_rows + p_rows
    H = T_ // 2
    assert (a_rows & (a_rows - 1)) == 0 and a_rows <= H

    nc = bacc.Bacc("TRN2", target_bir_lowering=False, debug=False)
    feats = nc.declare_dram_parameter("feats", [BL_, S_, T_], f32, isOutput=False)
    vs_out = nc.declare_dram_parameter("vs", [BL_, S_ - 1, T_], f32, isOutput=True)

    tblA = np.ascontiguousarray(trans_np[0:a_rows, :].reshape(1, a_rows * T_),
                                dtype=np.float32)
    tblA_d = nc.inline_tensor(tblA, "tblA")
    # shared compact transposed table: tDP[j, k] = trans[a + k, j]
    tblDP = np.ascontiguousarray(trans_np[a_rows:, :].T.reshape(1, T_ * dp),
                                 dtype=np.float32)
    tblDP_d = nc.inline_tensor(tblDP, "tblDP")

    with tile.TileContext(nc) as tc:
        with (
            tc.tile_pool(name="const", bufs=1) as cpool,
            tc.tile_pool(name="feat", bufs=2) as fpool,
            tc.tile_pool(name="vsb", bufs=2) as vspool,
            tc.tile_pool(name="sc", bufs=1) as scpool,
            tc.tile_pool(name="small", bufs=2) as smpool,
        ):
            tbA = cpool.tile([BL_, a_rows * T_], f32, tag="tbA")
            nc.gpsimd.dma_start(tbA[:, :], tblA_d[:, :].partition_broadcast(BL_))
            tbDP = cpool.tile([BL_, T_ * dp], f32, tag="tbDP")
            nc.gpsimd.dma_start(tbDP[:, :], tblDP_d[:, :].partition_broadcast(BL_))
            tbDP3 = tbDP[:, :].rearrange("p (j k) -> p j k", j=T_)

            sA = scpool.tile([BL_, a_rows * T_], f32, tag="sA")
            sA3 = sA[:, :].rearrange("p (i j) -> p i j", i=a_rows)
            sDP = scpool.tile([BL_, T_ * dp], f32, tag="sDP")
            sDP3 = sDP[:, :].rearrange("p (j k) -> p j k", j=T_)

            fb = fpool.tile([BL_, kblk * T_], f32, tag="fb")
            nc.gpsimd.dma_start(
                fb[:, :].rearrange("p (k t) -> p k t", k=kblk), feats[:, 0:kblk, :]
            )
            v = fb[:, 0:T_]  # v_0 = feats[:,0] (start folded by host)

            vsb = vspool.tile([BL_, kblk * T_], f32, tag="vsb")

            D0, D1 = a_rows, a_rows + d_rows
            for t in range(1, S_):
                kf, rf = divmod(t, kblk)
                if rf == 0:
                    fb = fpool.tile([BL_, kblk * T_], f32, tag="fb")
                    hi = min(kblk, S_ - kf * kblk)
                    nc.gpsimd.dma_start(
                        fb[:, 0 : hi * T_].rearrange("p (k t) -> p k t", k=hi),
                        feats[:, kf * kblk : kf * kblk + hi, :],
                    )
                ft = fb[:, rf * T_ : (rf + 1) * T_]

                # --- score adds (Act needs only vn half 0 of step t-1) ---
                for i in range(a_rows):
                    nc.scalar.add(
                        sA[:, i * T_ : (i + 1) * T_],
                        tbA[:, i * T_ : (i + 1) * T_],
                        v[:, i : i + 1],
                    )
                if d_rows:
                    vD = v[:, D0:D1].unsqueeze(1).broadcast_to([BL_, T_, d_rows])
                    nc.vector.scalar_tensor_tensor(
                        sDP3[:, :, 0:d_rows], tbDP3[:, :, 0:d_rows], 1.0, vD,
                        mul_op, add,
                    )
                vP = v[:, D1:T_].unsqueeze(1).broadcast_to([BL_, T_, p_rows])
                nc.gpsimd.tensor_tensor(
                    sDP3[:, :, d_rows:dp], tbDP3[:, :, d_rows:dp], vP, add
                )

                # --- maxes on DVE ---
                mxA = smpool.tile([BL_, T_], f32, tag="mxA")
                w = a_rows // 2
                while w >= 1:
                    i0 = sA3[:, 0:w, :]
                    i1 = sA3[:, w : 2 * w, :]
                    out = i0 if w > 1 else mxA[:, :].rearrange(
                        "p (i j) -> p i j", i=1
                    )
                    nc.vector.scalar_tensor_tensor(out, i0, 1.0, i1, mul_op, mx_op)
                    w //= 2

                r = (t - 1) % kblk
                if r == 0:
                    vsb = vspool.tile([BL_, kblk * T_], f32, tag="vsb")
                vslot = vsb[:, r * T_ : (r + 1) * T_]
                mxP = smpool.tile([BL_, T_], f32, tag="mxP")
                for h0, h1 in ((0, H), (H, T_)):
                    nc.vector.tensor_reduce(
                        mxP[:, h0:h1], sDP3[:, h0:h1, :], axis=X, op=mx_op
                    )
                    nc.vector.scalar_tensor_tensor(
                        mxA[:, h0:h1], mxA[:, h0:h1], 1.0, mxP[:, h0:h1],
                        mul_op, mx_op,
                    )
                    nc.vector.scalar_tensor_tensor(
                        vslot[:, h0:h1], mxA[:, h0:h1], 1.0, ft[:, h0:h1],
                        mul_op, add,
                    )
                v = vslot

                if r == kblk - 1 or t == S_ - 1:
                    lo = (t - 1) - r
                    n = r + 1
                    nc.gpsimd.dma_start(
                        vs_out[:, lo : lo + n, :],
                        vsb[:, 0 : n * T_].rearrange("p (k t) -> p k t", k=n),
                    )
    nc.finalize()
    return nc


def build_viterbi_v6_nc(trans_np, S_=S, T_=T, BL_=BL, kblk=8, jp=72,
                        pchunk=2):
    """j-split DVE/Pool kernel, fp32 exact, all-contiguous [j, i] layout.

    Per step, columns j of the score matrix s[b,j,i] = v[b,i] + tT[j,i] are
    split: DVE computes js = [0, T-jp) with one STT add, Pool computes
    [T-jp, T) in `pchunk` contiguous chunks into its own buffer. DVE owns
    every max: it reduces its own slice while Pool streams, then reduces
    Pool's chunks as they land, then vn = mx + feat. Pool's adds for step
    t+1 overlap DVE's reduce phase of step t only up to the vn dependency,
    so the period is max(DVE busy, Pool chain + last chunk reduce + vn).
    All reduces are contiguous (1.051 ns/elem) and every instruction waits
    on at most one foreign semaphore.
    """
    import concourse.bacc as bacc
    import concourse.mybir as mybir
    import concourse.tile as tile

    f32 = mybir.dt.float32
    add = mybir.AluOpType.add
    mx_op = mybir.AluOpType.max
    mul_op = mybir.AluOpType.mult
    X = mybir.AxisListType.X

    jd = T_ - jp
    # uneven chunks: equal big chunks + a smaller last chunk to shrink the
    # post-Pool tail (last-chunk reduce + vn sit on the critical chain)
    last = max(8, jp // (2 * pchunk))
    big = (jp - last) // (pchunk - 1) if pchunk > 1 else 0
    chunks = [big] * (pchunk - 1) + [jp - big * (pchunk - 1)] if pchunk > 1 else [jp]
    assert sum(chunks) == jp

    nc = bacc.Bacc("TRN2", target_bir_lowering=False, debug=False)
    feats = nc.declare_dram_parameter("feats", [BL_, S_, T_], f32, isOutput=False)
    vs_out = nc.declare_dram_parameter("vs", [BL_, S_ - 1, T_], f32, isOutput=True)

    tT = np.ascontiguousarray(trans_np.T, dtype=np.float32)  # [j, i]
    tD = np.ascontiguousarray(tT[0:jd].reshape(1, jd * T_))
    tP = np.ascontiguousarray(tT[jd:].reshape(1, jp * T_))
    tD_d = nc.inline_tensor(tD, "tD")
    tP_d = nc.inline_tensor(tP, "tP")

    with tile.TileContext(nc) as tc:
        with (
            tc.tile_pool(name="const", bufs=1) as cpool,
            tc.tile_pool(name="feat", bufs=2) as fpool,
            tc.tile_pool(name="vsb", bufs=2) as vspool,
            tc.tile_pool(name="sc", bufs=1) as scpool,
            tc.tile_pool(name="small", bufs=2) as smpool,
        ):
            tbD = cpool.tile([BL_, jd * T_], f32, tag="tbD")
            nc.gpsimd.dma_start(tbD[:, :], tD_d[:, :].partition_broadcast(BL_))
            tbD3 = tbD[:, :].rearrange("p (j i) -> p j i", j=jd)
            tbP = cpool.tile([BL_, jp * T_], f32, tag="tbP")
            nc.gpsimd.dma_start(tbP[:, :], tP_d[:, :].partition_broadcast(BL_))

            sD = scpool.tile([BL_, jd * T_], f32, tag="sD")
            sD3 = sD[:, :].rearrange("p (j i) -> p j i", j=jd)
            sP = scpool.tile([BL_, jp * T_], f32, tag="sP")

            fb = fpool.tile([BL_, kblk * T_], f32, tag="fb")
            nc.gpsimd.dma_start(
                fb[:, :].rearrange("p (k t) -> p k t", k=kblk), feats[:, 0:kblk, :]
            )
            v = fb[:, 0:T_]  # v_0 = feats[:,0] (start folded by host)

            vsb = vspool.tile([BL_, kblk * T_], f32, tag="vsb")

            for t in range(1, S_):
                kf, rf = divmod(t, kblk)
                if rf == 0:
                    fb = fpool.tile([BL_, kblk * T_], f32, tag="fb")
                    hi = min(kblk, S_ - kf * kblk)
                    nc.scalar.dma_start(
                        fb[:, 0 : hi * T_].rearrange("p (k t) -> p k t", k=hi),
                        feats[:, kf * kblk : kf * kblk + hi, :],
                    )
                ft = fb[:, rf * T_ : (rf + 1) * T_]

                mxt = smpool.tile([BL_, T_], f32, tag="mxt")

                # Pool: its j-slice in contiguous chunks (own buffer)
                off = 0
                for c, w in enumerate(chunks):
                    lo, hi_ = off, off + w
                    off = hi_
                    jc = w
                    vC = v[:, :].unsqueeze(1).broadcast_to([BL_, jc, T_])
                    nc.gpsimd.tensor_tensor(
                        sP[:, lo * T_ : hi_ * T_].rearrange(
                            "p (j i) -> p j i", j=jc
                        ),
                        tbP[:, lo * T_ : hi_ * T_].rearrange(
                            "p (j i) -> p j i", j=jc
                        ),
                        vC,
                        add,
                    )

                # DVE: own slice add + reduce, then Pool-chunk reduces
                vD = v[:, :].unsqueeze(1).broadcast_to([BL_, jd, T_])
                nc.vector.scalar_tensor_tensor(sD3, vD, 1.0, tbD3, mul_op, add)
                nc.vector.tensor_reduce(mxt[:, 0:jd], sD3, axis=X, op=mx_op)
                off = 0
                for c, w in enumerate(chunks):
                    lo, hi_ = off, off + w
                    off = hi_
                    nc.vector.tensor_reduce(
                        mxt[:, jd + lo : jd + hi_],
                        sP[:, lo * T_ : hi_ * T_].rearrange(
                            "p (j i) -> p j i", j=w
                        ),
                        axis=X,
                        op=mx_op,
                    )

                r = (t - 1) % kblk
                if r == 0:
                    vsb = vspool.tile([BL_, kblk * T_], f32, tag="vsb")
                vslot = vsb[:, r * T_ : (r + 1) * T_]
                nc.vector.scalar_tensor_tensor(
                    vslot, mxt[:, :], 1.0, ft, mul_op, add
                )
                v = vslot

                if r == kblk - 1 or t == S_ - 1:
                    lo = (t - 1) - r
                    n = r + 1
                    nc.scalar.dma_start(
                        vs_out[:, lo : lo + n, :],
                        vsb[:, 0 : n * T_].rearrange("p (k t) -> p k t", k=n),
                    )
    nc.finalize()
    return nc


def build_viterbi_v7_nc(trans_np, S_=S, T_=T, BL_=BL, kblk=4, jp=86,
                        pchunk=4):
    """Bidirectional j-split kernel: forward chain (t = 1..tau) and backward
    chain (t = S-2..tau) interleaved, tau = S//2. The chains are data-
    independent, so Pool computes one chain's score adds while DVE reduces
    the other's — removing the add/reduce alternation stall of the
    unidirectional kernels.

    fwd:  v_t[j] = max_i(v[i] + tT[j,i]) + feat_t[j]         ([j,i] table)
    bwd:  h = feat'_{t+1} + G_{t+1};  G_t[i] = max_j(h[j] + t[i,j])
          ([i,j] table; feat'[S-1] has stop folded, G_{S-1} = 0)
    Host decodes [0..tau] from the v stream, picks tag_tau =
    argmax(v_tau + G_tau), and forward-traces [tau..S-1] from the G stream.

    Tables are stored fp16 (halves SBUF so both chains fit); scores are
    fp32 with a single rounding, so only the table quantization perturbs
    results (measured harmless). Pool writes its j-chunks into bufs=2
    chunk tiles; DVE reduces each chunk as it lands.
    """
    import concourse.bacc as bacc
    import concourse.mybir as mybir
    import concourse.tile as tile

    f32 = mybir.dt.float32
    f16 = mybir.dt.float16
    add = mybir.AluOpType.add
    mx_op = mybir.AluOpType.max
    mul_op = mybir.AluOpType.mult
    X = mybir.AxisListType.X

    jd = T_ - jp
    tau = S_ // 2
    nF = tau          # fwd steps t = 1..tau, stream rows 0..nF-1
    nB = S_ - 1 - tau  # bwd steps t = S-2..tau, stream rows 0..nB-1

    # chunk widths: equal-ish with a smaller last chunk
    last = max(8, jp // (2 * pchunk))
    big = (jp - last) // (pchunk - 1) if pchunk > 1 else 0
    chunks = [big] * (pchunk - 1) + [jp - big * (pchunk - 1)] if pchunk > 1 else [jp]

    nc = bacc.Bacc("TRN2", target_bir_lowering=False, debug=False)
    feats = nc.declare_dram_parameter("feats", [BL_, S_, T_], f32, isOutput=False)
    vsF = nc.declare_dram_parameter("vsF", [BL_, nF, T_], f32, isOutput=True)
    vsB = nc.declare_dram_parameter("vsB", [BL_, nB, T_], f32, isOutput=True)

    t16 = trans_np.astype(np.float16)
    tTf = np.ascontiguousarray(t16.T.reshape(1, T_ * T_))  # [j, i] for fwd
    tPf = np.ascontiguousarray(t16.reshape(1, T_ * T_))    # [i, j] for bwd
    tT_d = nc.inline_tensor(tTf, "tTf")
    tP_d = nc.inline_tensor(tPf, "tPf")

    with tile.TileContext(nc) as tc:
        with (
            tc.tile_pool(name="const", bufs=1) as cpool,
            tc.tile_pool(name="featF", bufs=2) as fFpool,
            tc.tile_pool(name="featB", bufs=2) as fBpool,
            tc.tile_pool(name="vsbF", bufs=2) as vFpool,
            tc.tile_pool(name="vsbB", bufs=2) as vBpool,
            tc.tile_pool(name="scD", bufs=1) as sdpool,
            tc.tile_pool(name="scPF", bufs=2) as spFpool,
            tc.tile_pool(name="scPB", bufs=2) as spBpool,
            tc.tile_pool(name="small", bufs=3) as smpool,
        ):
            tbF = cpool.tile([BL_, T_ * T_], f16, tag="tbF")
            nc.gpsimd.dma_start(tbF[:, :], tT_d[:, :].partition_broadcast(BL_))
            tbB = cpool.tile([BL_, T_ * T_], f16, tag="tbB")
            nc.gpsimd.dma_start(tbB[:, :], tP_d[:, :].partition_broadcast(BL_))

            sDF = sdpool.tile([BL_, jd * T_], f32, tag="sDF")
            sDF3 = sDF[:, :].rearrange("p (j i) -> p j i", j=jd)
            sDB = sdpool.tile([BL_, jd * T_], f32, tag="sDB")
            sDB3 = sDB[:, :].rearrange("p (j i) -> p j i", j=jd)

            fbF = fFpool.tile([BL_, kblk * T_], f32, tag="fbF")
            nc.scalar.dma_start(
                fbF[:, :].rearrange("p (k t) -> p k t", k=kblk),
                feats[:, 0:kblk, :],
            )
            qb0 = S_ - kblk
            fbB = fBpool.tile([BL_, kblk * T_], f32, tag="fbB")
            nc.scalar.dma_start(
                fbB[:, :].rearrange("p (k t) -> p k t", k=kblk),
                feats[:, qb0:S_, :],
            )

            vF = fbF[:, 0:T_]   # v_0 (start folded by host)
            hB = fbB[:, (S_ - 1 - qb0) * T_ : (S_ - qb0) * T_]  # feat'_{S-1}

            vsFb = vFpool.tile([BL_, kblk * T_], f32, tag="vsFb")
            vsBb = vBpool.tile([BL_, kblk * T_], f32, tag="vsBb")

            def chain_step(tbl, sD3, vsrc, mxt):
                """one j-split step: helpers+DVE adds, DVE reduces into mxt"""
                tb3d = tbl[:, 0 : jd * T_].rearrange("p (j i) -> p j i", j=jd)
                vD = vsrc.unsqueeze(1).broadcast_to([BL_, jd, T_])
                nc.vector.scalar_tensor_tensor(sD3, tb3d, 1.0, vD, mul_op, add)
                nc.vector.tensor_reduce(mxt[:, 0:jd], sD3, axis=X, op=mx_op)

            for k in range(nF):
                # ---------- forward step t = 1 + k ----------
                t = 1 + k
                kf, rf = divmod(t, kblk)
                if rf == 0:
                    fbF = fFpool.tile([BL_, kblk * T_], f32, tag="fbF")
                    hi = min(kblk, S_ - kf * kblk)
                    nc.scalar.dma_start(
                        fbF[:, 0 : hi * T_].rearrange("p (k t) -> p k t", k=hi),
                        feats[:, kf * kblk : kf * kblk + hi, :],
                    )
                ftF = fbF[:, rf * T_ : (rf + 1) * T_]

                mxF = smpool.tile([BL_, T_], f32, tag="mxF")
                # Pool chunks for fwd
                pf_tiles = []
                off = jd
                for w in chunks:
                    sPF = spFpool.tile([BL_, w * T_], f32, tag="sPF")
                    vC = vF[:, :].unsqueeze(1).broadcast_to([BL_, w, T_])
                    nc.gpsimd.tensor_tensor(
                        sPF[:, :].rearrange("p (j i) -> p j i", j=w),
                        tbF[:, off * T_ : (off + w) * T_].rearrange(
                            "p (j i) -> p j i", j=w
                        ),
                        vC,
                        add,
                    )
                    pf_tiles.append((sPF, off, w))
                    off += w
                chain_step(tbF, sDF3, vF[:, :], mxF)
                for sPF, off_, w in pf_tiles:
                    nc.vector.tensor_reduce(
                        mxF[:, off_ : off_ + w],
                        sPF[:, :].rearrange("p (j i) -> p j i", j=w),
                        axis=X,
                        op=mx_op,
                    )
                r = t - 1  # stream row
                if r % kblk == 0:
                    vsFb = vFpool.tile([BL_, kblk * T_], f32, tag="vsFb")
                vslotF = vsFb[:, (r % kblk) * T_ : (r % kblk + 1) * T_]
                nc.vector.scalar_tensor_tensor(
                    vslotF, mxF[:, :], 1.0, ftF, mul_op, add
                )
                vF = vslotF
                if r % kblk == kblk - 1 or r == nF - 1:
                    lo = r - (r % kblk)
                    n = (r % kblk) + 1
                    nc.scalar.dma_start(
                        vsF[:, lo : lo + n, :],
                        vsFb[:, 0 : n * T_].rearrange("p (k t) -> p k t", k=n),
                    )

                # ---------- backward step t = S-2-k (if any) ----------
                if k < nB:
                    t_b = S_ - 2 - k
                    q = t_b + 1  # feat'_{t+1} index, descending from S-1
                    mxB = smpool.tile([BL_, T_], f32, tag="mxB")
                    pb_tiles = []
                    off = jd
                    for w in chunks:
                        sPB = spBpool.tile([BL_, w * T_], f32, tag="sPB")
                        hC = hB.unsqueeze(1).broadcast_to([BL_, w, T_])
                        nc.gpsimd.tensor_tensor(
                            sPB[:, :].rearrange("p (j i) -> p j i", j=w),
                            tbB[:, off * T_ : (off + w) * T_].rearrange(
                                "p (j i) -> p j i", j=w
                            ),
                            hC,
                            add,
                        )
                        pb_tiles.append((sPB, off, w))
                        off += w
                    chain_step(tbB, sDB3, hB, mxB)
                    for sPB, off_, w in pb_tiles:
                        nc.vector.tensor_reduce(
                            mxB[:, off_ : off_ + w],
                            sPB[:, :].rearrange("p (j i) -> p j i", j=w),
                            axis=X,
                            op=mx_op,
                        )
                    # G_t = mxB; stream row m = t_b - tau descending
                    m = t_b - tau
                    if m % kblk == kblk - 1 or m == nB - 1:
                        vsBb = vBpool.tile([BL_, kblk * T_], f32, tag="vsBb")
                    gslot = vsBb[:, (m % kblk) * T_ : (m % kblk + 1) * T_]
                    nc.vector.tensor_copy(gslot, mxB[:, :])
                    if m % kblk == 0:
                        n = kblk if (m + kblk <= nB) else (nB - m)
                        # rows [m, m+n) are in the buffer (written descending)
                        nc.scalar.dma_start(
                            vsB[:, m : m + n, :],
                            vsBb[:, 0 : n * T_].rearrange(
                                "p (k t) -> p k t", k=n
                            ),
                        )
                    # next h = feat'_{t_b} + G_{t_b}  (for the NEXT bwd step)
                    if k + 1 < nB:
                        qn = t_b  # next step's feat index
                        if qn % kblk == kblk - 1:
                            fbB = fBpool.tile([BL_, kblk * T_], f32, tag="fbB")
                            lo_q = qn - (kblk - 1)
                            nc.scalar.dma_start(
                                fbB[:, :].rearrange("p (k t) -> p k t", k=kblk),
                                feats[:, lo_q : lo_q + kblk, :],
                            )
                            qblo = lo_q
                        else:
                            qblo = qn - (qn % kblk)
                        hnew = smpool.tile([BL_, T_], f32, tag="hB")
                        nc.vector.scalar_tensor_tensor(
                            hnew[:, :],
                            mxB[:, :],
                            1.0,
                            fbB[:, (qn - qblo) * T_ : (qn - qblo + 1) * T_],
                            mul_op,
                            add,
                        )
                        hB = hnew[:, :]
    nc.finalize()
    return nc


def build_viterbi_v8_nc(trans_np, S_=S, T_=T, BL_=BL, kblk=8, a_rows=48,
                        jp=80):
    """Hybrid row+column split, fp32, all big instructions.

    Score s[b,i,j] = t[i,j] + v[b,i]:
      - Act: rows i in [0, a)  -> sA ([i, j] row-contig), one activation-add
        per row; DVE max-tree over the rows -> mxA[j].
      - remaining rows i in [a, T) as a compact transposed block
        c[b, j, k] = v[a+k] + tC[j, k] (tC[j,k] = t[a+k, j]):
          Pool: columns j in [T-jp, T) as ONE tensor_tensor (big Pool
          instructions amortize its ~4-5us fixed overhead),
          DVE: columns j in [0, T-jp) with one STT.
        DVE reduces both column blocks contiguously -> mxt[j].
      - vn = max(mxt, mxA) + feat -> vs slot (b-major stream).
    Critical chains (all ~equal by construction):
      vn -> Act rows -> tree -> vn   and   vn -> Pool add -> reduceP -> vn.
    """
    import concourse.bacc as bacc
    import concourse.mybir as mybir
    import concourse.tile as tile

    f32 = mybir.dt.float32
    add = mybir.AluOpType.add
    mx_op = mybir.AluOpType.max
    mul_op = mybir.AluOpType.mult
    X = mybir.AxisListType.X

    K = T_ - a_rows      # compact width
    jdv = T_ - jp        # DVE's column share
    jpe = jp + 1         # Pool duplicates column jdv-1: its reduce output
                         # region then overlaps the tree-combine's, forcing
                         # the scheduler to order the tree BEFORE reduceP

    nc = bacc.Bacc("TRN2", target_bir_lowering=False, debug=False)
    feats = nc.declare_dram_parameter("feats", [BL_, S_, T_], f32, isOutput=False)
    vs_out = nc.declare_dram_parameter("vs", [BL_, S_ - 1, T_], f32, isOutput=True)

    tA = np.ascontiguousarray(trans_np[0:a_rows, :].reshape(1, a_rows * T_),
                              dtype=np.float32)
    tA_d = nc.inline_tensor(tA, "tA")
    tC = np.ascontiguousarray(trans_np[a_rows:, :].T.reshape(1, T_ * K),
                              dtype=np.float32)
    tC_d = nc.inline_tensor(tC, "tC")

    with tile.TileContext(nc) as tc:
        with (
            tc.tile_pool(name="const", bufs=1) as cpool,
            tc.tile_pool(name="feat", bufs=2) as fpool,
            tc.tile_pool(name="vsb", bufs=2) as vspool,
            tc.tile_pool(name="sc", bufs=1) as scpool,
            tc.tile_pool(name="small", bufs=3) as smpool,
        ):
            tbA = cpool.tile([BL_, a_rows * T_], f32, tag="tbA")
            nc.gpsimd.dma_start(tbA[:, :], tA_d[:, :].partition_broadcast(BL_))
            tbC = cpool.tile([BL_, T_ * K], f32, tag="tbC")
            nc.gpsimd.dma_start(tbC[:, :], tC_d[:, :].partition_broadcast(BL_))
            tbC3 = tbC[:, :].rearrange("p (j k) -> p j k", j=T_)

            sA = scpool.tile([BL_, a_rows * T_], f32, tag="sA")
            sA3 = sA[:, :].rearrange("p (i j) -> p i j", i=a_rows)
            sD = scpool.tile([BL_, jdv * K], f32, tag="sD")
            sD3 = sD[:, :].rearrange("p (j k) -> p j k", j=jdv)
            sP = scpool.tile([BL_, jpe * K], f32, tag="sP")
            sP3 = sP[:, :].rearrange("p (j k) -> p j k", j=jpe)

            fb = fpool.tile([BL_, kblk * T_], f32, tag="fb")
            nc.scalar.dma_start(
                fb[:, :].rearrange("p (k t) -> p k t", k=kblk), feats[:, 0:kblk, :]
            )
            v = fb[:, 0:T_]  # v_0 (start folded by host)
            vsb = vspool.tile([BL_, kblk * T_], f32, tag="vsb")

            for t in range(1, S_):
                kf, rf = divmod(t, kblk)
                if rf == 0:
                    fb = fpool.tile([BL_, kblk * T_], f32, tag="fb")
                    hi = min(kblk, S_ - kf * kblk)
                    nc.scalar.dma_start(
                        fb[:, 0 : hi * T_].rearrange("p (k t) -> p k t", k=hi),
                        feats[:, kf * kblk : kf * kblk + hi, :],
                    )
                ft = fb[:, rf * T_ : (rf + 1) * T_]

                # Act rows
                for i in range(a_rows):
                    nc.scalar.add(
                        sA[:, i * T_ : (i + 1) * T_],
                        tbA[:, i * T_ : (i + 1) * T_],
                        v[:, i : i + 1],
                    )
                # Pool: one big TT for its column block
                vv = v[:, a_rows:T_]
                vP = vv.unsqueeze(1).broadcast_to([BL_, jpe, K])
                nc.gpsimd.tensor_tensor(sP3, tbC3[:, jdv - 1 : T_, :], vP, add)
                # DVE: its column block
                vD = vv.unsqueeze(1).broadcast_to([BL_, jdv, K])
                nc.vector.scalar_tensor_tensor(
                    sD3, tbC3[:, 0:jdv, :], 1.0, vD, mul_op, add
                )

                mxt = smpool.tile([BL_, T_], f32, tag="mxt")
                nc.vector.tensor_reduce(mxt[:, 0:jdv], sD3, axis=X, op=mx_op)

                # max tree over Act rows (48 = 3 * 16): halve to 3, then fold
                mxA = smpool.tile([BL_, T_], f32, tag="mxA")
                w = a_rows // 2
                while w >= 3:
                    nc.vector.scalar_tensor_tensor(
                        sA3[:, 0:w, :], sA3[:, 0:w, :], 1.0,
                        sA3[:, w : 2 * w, :], mul_op, mx_op,
                    )
                    w //= 2
                # rows 0,1,2 remain
                nc.vector.scalar_tensor_tensor(
                    sA3[:, 0:1, :], sA3[:, 0:1, :], 1.0, sA3[:, 1:2, :],
                    mul_op, mx_op,
                )
                nc.vector.scalar_tensor_tensor(
                    mxA[:, :].rearrange("p (i j) -> p i j", i=1),
                    sA3[:, 0:1, :], 1.0, sA3[:, 2:3, :], mul_op, mx_op,
                )

                nc.vector.tensor_reduce(
                    mxt[:, jdv - 1 : T_], sP3, axis=X, op=mx_op
                )
                nc.vector.scalar_tensor_tensor(
                    mxt[:, :], mxt[:, :], 1.0, mxA[:, :], mul_op, mx_op
                )
                r = (t - 1) % kblk
                if r == 0:
                    vsb = vspool.tile([BL_, kblk * T_], f32, tag="vsb")
                vslot = vsb[:, r * T_ : (r + 1) * T_]
                nc.vector.scalar_tensor_tensor(
                    vslot, mxt[:, :], 1.0, ft, mul_op, add
                )
                v = vslot

                if r == kblk - 1 or t == S_ - 1:
                    lo = (t - 1) - r
                    n = r + 1
                    nc.scalar.dma_start(
                        vs_out[:, lo : lo + n, :],
                        vsb[:, 0 : n * T_].rearrange("p (k t) -> p k t", k=n),
                    )
    nc.finalize()
    return nc


def _install_ntff_hook_shim():
    """The agent image's `antenv` lacks `axon_hooks`, so trn_boot degrades
    silently and bass_utils' trace path crashes on import. Provide the same
    ctypes-based NTFF hook trn_boot would have registered."""
    import sys
    import types

    if "antenv.axon_hooks" in sys.modules:
        return
    try:
        import antenv.axon_hooks  # noqa: F401
        return
    except ImportError:
        pass
    try:
        from trn_agent_boot.trn_boot import _ntff_profile_via_ctypes

        hook = _ntff_profile_via_ctypes("/opt/axon/libaxon_pjrt.so")
    except Exception:
        hook = None
    m = types.ModuleType("antenv.axon_hooks")
    m._hook = hook
    m.get_axon_ntff_profile_hook = lambda: m._hook
    def _set(h):
        m._hook = h
    m.set_axon_ntff_profile_hook = _set
    sys.modules["antenv.axon_hooks"] = m


def _run(nc, in_maps, **kwargs):
    if kwargs.get("trace"):
        _install_ntff_hook_shim()
    from concourse.bass_utils import run_bass_kernel_spmd

    return run_bass_kernel_spmd(nc, in_maps, core_ids=list(range(len(in_maps))), **kwargs)


def _backtrace_from_vs(vs, v0, trans, stop):
    """Exact backtrace from per-step state vectors.

    vs: [B, S-1, T] fp32 (v at t=1..S-1), v0: [B, T] (v at t=0).
    Recomputes argmax_i(v[t-1,:,i] + trans[i, j_t]) along the traced path
    only — identical fp32 arithmetic + first-index ties as the reference.
    """
    B_, Sm1, T_ = vs.shape
    S_ = Sm1 + 1
    last = np.argmax(vs[:, -1, :] + stop[None, :], axis=1).astype(np.int32)
    tags = np.empty((B_, S_), dtype=np.int32)
    tags[:, -1] = last
    cur = last
    transT = np.ascontiguousarray(trans.T)  # [j, i]
    for t in range(S_ - 1, 0, -1):
        vprev = vs[:, t - 2, :] if t >= 2 else v0
        col = vprev + transT[cur]  # [B, T] fp32: v[b,t-1,i] + trans[i, j_t]
        cur = np.argmax(col, axis=1).astype(np.int32)
        tags[:, t - 1] = cur
    return tags


def kernel(feats, transitions, start_transitions, stop_transitions, _trace=False,
           _variant="v8"):
    feats = np.asarray(feats, dtype=np.float32).copy()
    trans = np.ascontiguousarray(np.asarray(transitions, dtype=np.float32))
    start = np.ascontiguousarray(np.asarray(start_transitions, dtype=np.float32))
    stop = np.ascontiguousarray(np.asarray(stop_transitions, dtype=np.float32))
    assert feats.shape == (B, S, T)

    feats[:, 0, :] += start  # fold start_transitions (bit-exact vs reference)

    if _variant == "v8":
        import os as _os
        nc = build_viterbi_v8_nc(
            trans,
            a_rows=int(_os.environ.get("VT_AROWS", "48")),
            jp=int(_os.environ.get("VT_JP8", "80")),
            kblk=int(_os.environ.get("VT_KBLK", "8")),
        )
    elif _variant == "v7":
        import os as _os
        feats[:, S - 1, :] += stop  # fold stop for the backward chain
        nc = build_viterbi_v7_nc(
            trans,
            jp=int(_os.environ.get("VT_JP", "86")),
            pchunk=int(_os.environ.get("VT_PCHUNK", "4")),
            kblk=int(_os.environ.get("VT_KBLK", "4")),
        )
    elif _variant == "v6":
        import os as _os
        nc = build_viterbi_v6_nc(
            trans,
            jp=int(_os.environ.get("VT_JP", "72")),
            pchunk=int(_os.environ.get("VT_PCHUNK", "3")),
            kblk=int(_os.environ.get("VT_KBLK", "8")),
        )
    elif _variant == "v5":
        import os as _os
        nc = build_viterbi_v5_nc(
            trans,
            a_rows=int(_os.environ.get("VT_AROWS", "32")),
            d_rows=int(_os.environ.get("VT_DROWS", "16")),
            kblk=int(_os.environ.get("VT_KBLK", "8")),
        )
    elif _variant == "v4":
        import os as _os
        nc = build_viterbi_v4_nc(
            trans,
            a_rows=int(_os.environ.get("VT_AROWS", "58")),
            kblk=int(_os.environ.get("VT_KBLK", "8")),
        )
    elif _variant == "f16":
        nc = build_viterbi_f16_nc(trans)
    else:
        nc = build_viterbi_nc(trans, variant=_variant)
    in_maps = [{"feats": feats[c * BL : (c + 1) * BL]} for c in range(NCORES)]
    res = _run(nc, in_maps, trace=_trace)

    if _variant == "v7":
        tau = S // 2
        vsF = np.concatenate([r["vsF"] for r in res.results], axis=0)  # [B, tau, T]
        vsB = np.concatenate([r["vsB"] for r in res.results], axis=0)  # [B, S-1-tau, T]
        t16 = trans.astype(np.float16).astype(np.float32)
        t16T = np.ascontiguousarray(t16.T)
        v0 = feats[:, 0, :]  # start folded
        tags = np.empty((B, S), dtype=np.int32)
        cur = np.argmax(vsF[:, tau - 1] + vsB[:, 0], axis=1).astype(np.int32)
        tags[:, tau] = cur
        # forward segment [0..tau-1]: same backtrace as before, fp16 table
        for t in range(tau, 0, -1):
            vprev = vsF[:, t - 2, :] if t >= 2 else v0
            cur = np.argmax(vprev + t16T[cur], axis=1).astype(np.int32)
            tags[:, t - 1] = cur
        # backward segment [tau+1..S-1]: forward-trace on the G stream
        cur = tags[:, tau].copy()
        for t in range(tau, S - 1):
            q = t + 1
            h = feats[:, q, :].copy()  # stop already folded into feats[S-1]
            if q <= S - 2:
                h = h + vsB[:, q - tau, :]
            cur = np.argmax(t16[cur] + h, axis=1).astype(np.int32)
            tags[:, q] = cur
    elif _variant in ("f16", "v4", "v5", "v6", "v8"):
        vs = np.concatenate([r["vs"] for r in res.results], axis=0)  # [B, S-1, T]
        v0 = feats[:, 0, :]  # start already folded
        tags = _backtrace_from_vs(vs, v0, trans, stop)
    elif _variant == "v0":
        bp_f = np.concatenate(
            [np.transpose(r["bp"], (1, 0, 2)) for r in res.results], axis=0
        )
        v_fin = np.concatenate([r["v_final"] for r in res.results], axis=0)
        idx = (T - 1) - bp_f.astype(np.int32)
        last = np.argmax(v_fin + stop[None, :], axis=1).astype(np.int32)
        tags = np.empty((B, S), dtype=np.int32)
        tags[:, S - 1] = last
        cur = last
        ar = np.arange(B)
        for t in range(S - 2, -1, -1):
            cur = idx[ar, t, cur]
            tags[:, t] = cur
    else:
        vs = np.concatenate(
            [np.transpose(r["vs"], (1, 0, 2)) for r in res.results], axis=0
        )  # [B, S-1, T]
        v0 = feats[:, 0, :]  # start already folded
        tags = _backtrace_from_vs(vs, v0, trans, stop)

    if _trace:
        return tags, res
    return tags

